# revision 1
# baseline (speedup 1.0000x reference)
"""Trainium2 Bass kernel for nn_MultiHeadSSAN: banded Q/K (prefix-sum windows
along feature_len) + multi-head self-attention, sharded over the feature_len
(L) axis across 8 NeuronCores.

Per-core plan (core k owns n in [CH*k, CH*(k+1))):
  Band:  Q[s,t,e] = x + (C1[t-1]-C1[t-n1]) + (C2[min(t+n2-1,L-1)]-C2[t]) with
         C1=cumsum(x*a), C2=cumsum(x*c) along L.  Computed as gated segmented
         scans (tensor_tensor_scan) over (s-sub x l) streams per e-block, on
         the own chunk plus one partner chunk (k -/+ OFF, host-prepared with
         sign/shift so the SPMD program is uniform).  Chunk-total boundary
         constants are AllGathered and folded into the q/k projections via an
         identity-matmul PSUM accumulate.
  MHA:   per n: q/k projections + both score orientations in fp32 on the PE;
         V-path in fp16.  Softmax subtracts lse = max + log(den) via a rank-1
         (K=1) matmul into the transposed-score PSUM, so exp() emits
         normalized attention directly (no reciprocal, no transposes).

DMA layouts obey: SBUF partition dim <-> strided DRAM dim, SBUF free dim <->
contiguous DRAM runs (>=512B where possible).  x is shipped in two layouts
(band: (E,S,CH); attention: (E,CH,S)); Q/K bounce through DRAM as (E,CH,S).
"""
import math
import numpy as np

import concourse.bass as bass
import concourse.bacc as bacc
import concourse.mybir as mybir
import concourse.tile as tile
from concourse.bass_utils import run_bass_kernel_spmd

F32 = mybir.dt.float32
BF16 = mybir.dt.bfloat16
F16 = mybir.dt.float16
ALU = mybir.AluOpType
ACTF = mybir.ActivationFunctionType
AX = mybir.AxisListType


class Cfg:
    def __init__(self, S=256, L=512, E=512, H=4, NC=8, OFF=4, SB=8,
                 v_dtype="fp16", no_collective=False, reps=1,
                 skip_band=False, skip_attn=False, nmax=None, tune=None):
        self.S, self.L, self.E, self.H, self.NC = S, L, E, H, NC
        self.CH = L // NC              # L-chunk per core
        self.OFF = OFF                 # partner offset = n1 // CH
        assert OFF * 2 >= NC, "single-partner scheme needs OFF >= NC/2"
        self.n1 = self.n2 = OFF * self.CH
        self.HD = E // H
        assert self.HD == 128, "head dim must be 128"
        assert E % 128 == 0
        self.EB = E // 128             # e partition blocks
        self.SB = SB                   # band s-sub size
        assert S % SB == 0
        self.NST = (S + 127) // 128    # s tiles of <=128 in phase D
        self.STW = min(128, S)         # s tile width
        self.v_dtype = v_dtype
        self.no_collective = no_collective
        self.reps = reps
        self.skip_band = skip_band
        self.skip_attn = skip_attn
        self.nmax = nmax if nmax is not None else self.CH
        self.tune = dict(ps_a=3, ps_b=3, ps_t=2, dpool=2, scan=5, prod=2, qkp=12, PT=10)
        if tune: self.tune.update(tune)

    def key(self):
        return (self.S, self.L, self.E, self.H, self.NC, self.OFF, self.SB,
                self.v_dtype, self.no_collective, self.reps,
                self.skip_band, self.skip_attn, self.nmax,
                tuple(sorted(self.tune.items())))


def build_nc(cfg: Cfg) -> bass.Bass:
    S, L, E, H, NC = cfg.S, cfg.L, cfg.E, cfg.H, cfg.NC
    CH, EB, SB, HD = cfg.CH, cfg.EB, cfg.SB, cfg.HD
    NSS = S // SB                      # band s-sub count
    BW = SB * CH                       # band tile free width
    NST, STW = cfg.NST, cfg.STW
    VDT = F16 if cfg.v_dtype == "fp16" else None
    NPAIR = 2 if CH % 2 == 0 else 1    # phase-D n's loaded per DMA

    nc = bacc.Bacc(None)
    # ---- parameters (layouts chosen for contiguous per-partition DMA runs)
    xband = nc.declare_dram_parameter("xband", [E, S, CH], F32, isOutput=False)
    xattn = nc.declare_dram_parameter("xattn", [E, CH, S], F32, isOutput=False)
    xp = nc.declare_dram_parameter("xp", [E, S, CH], F32, isOutput=False)
    wband = nc.declare_dram_parameter("wband", [6, E, CH], F32, isOutput=False)
    gate_in = nc.declare_dram_parameter("gate_in", [128, BW], F32, isOutput=False)
    coef = nc.declare_dram_parameter("coef", [128, 2 * NC], F32, isOutput=False)
    wq = nc.declare_dram_parameter("wq", [E, E], F32, isOutput=False)
    wk = nc.declare_dram_parameter("wk", [E, E], F32, isOutput=False)
    wv = nc.declare_dram_parameter("wv", [E, E], F16 if VDT else F32, isOutput=False)
    wo = nc.declare_dram_parameter("wo", [E, E], F16 if VDT else F32, isOutput=False)
    biasr = nc.declare_dram_parameter("biasr", [4, E], F32, isOutput=False)
    biasc = nc.declare_dram_parameter("biasc", [E, 4], F32, isOutput=False)
    ident_in = nc.declare_dram_parameter("ident_in", [128, 128], F32, isOutput=False)
    out = nc.declare_dram_parameter("out", [E, CH, S], F32, isOutput=True)

    # ---- internal DRAM
    qdram = nc.dram_tensor("qdram", [E, CH + 1, S], F32)
    kdram = nc.dram_tensor("kdram", [E, CH + 1, S], F32)
    tin = nc.dram_tensor("tin", [4, E, S], F32)
    tout = nc.dram_tensor("tout", [4 * NC, E, S], F32, addr_space="Shared")

    with tile.TileContext(nc) as tc:
        with (
            tc.tile_pool(name="const", bufs=1) as cpool,
            tc.tile_pool(name="band", bufs=2) as bpool,
            tc.tile_pool(name="scan", bufs=6) as spool,
            tc.tile_pool(name="bc", bufs=2) as bcpool,
            tc.tile_pool(name="dpool", bufs=cfg.tune["dpool"]) as dpool,
            tc.tile_pool(name="evac", bufs=3) as epool,
            tc.tile_pool(name="ps_a", bufs=cfg.tune["ps_a"], space="PSUM") as ps_a,
            tc.tile_pool(name="ps_b", bufs=cfg.tune["ps_b"], space="PSUM") as ps_b,
            tc.tile_pool(name="ps_t", bufs=cfg.tune["ps_t"], space="PSUM") as ps_t,
            tc.tile_pool(name="dbounce", bufs=4, space="DRAM") as dbpool,
        ):
            # ================= setup =================
            gate = cpool.tile([128, BW], F32, name="gate")
            nc.sync.dma_start(gate[:], gate_in[:, :])
            ident = cpool.tile([128, 128], F32, name="ident")
            nc.sync.dma_start(ident[:], ident_in[:, :])
            coef_sb = cpool.tile([128, 2 * NC], F32, name="coef_sb")
            nc.sync.dma_start(coef_sb[:], coef[:, :])
            biasrow = []
            for j in range(4):
                t = cpool.tile([1, E], F32, name=f"biasrow{j}")
                nc.sync.dma_start(t[:], biasr[j:j + 1, :])
                biasrow.append(t)
            biasc_sb = cpool.tile([128, 4 * EB], F32, name="biasc_sb")
            for eb in range(EB):
                nc.sync.dma_start(biasc_sb[:, 4 * eb:4 * (eb + 1)],
                                  biasc[eb * 128:(eb + 1) * 128, :])
            ones_row = cpool.tile([1, max(S, 128)], F32, name="ones_row")
            nc.vector.memset(ones_row[:], 1.0)

            wband_sb = []
            for kind in range(6):
                row = []
                for eb in range(EB):
                    t = cpool.tile([128, CH], F32, name=f"wband_{kind}_{eb}")
                    nc.sync.dma_start(t[:], wband[kind, eb * 128:(eb + 1) * 128, :])
                    row.append(t)
                wband_sb.append(row)

            def load_w(dram, nm, dt):
                tiles = []
                for eb in range(EB):
                    t = cpool.tile([128, E], dt, name=f"{nm}_{eb}")
                    nc.sync.dma_start(t[:], dram[eb * 128:(eb + 1) * 128, :])
                    tiles.append(t)
                return tiles

            wq_sb = load_w(wq, "wq", F32)
            wk_sb = load_w(wk, "wk", F32)
            wv_sb = load_w(wv, "wv", F16 if VDT else F32)
            wo_v = load_w(wo, "wo", F16 if VDT else F32)

            def emit_attn(n, qt, kt, xth, Bqp, Bkp):
                # q/k projections: (f, s) tiles
                def proj(w_sb, src, Bp, nm):
                    outt = []
                    for fm in range(EB):
                        fr = slice(fm * 128, (fm + 1) * 128)
                        acc = ps_a.tile([128, S], F32, name=f"ps{nm}{fm}", tag="ps_mm")
                        for eb in range(EB):
                            nc.tensor.matmul(acc[:], w_sb[eb][:, fr], src[eb],
                                             start=(eb == 0), stop=False)
                        nc.tensor.matmul(acc[:], ident[:], Bp[fm][:],
                                         start=False, stop=True)
                        o = epool.tile([128, S], F32, name=f"{nm}_{fm}", tag="qkp",
                                       bufs=cfg.tune["qkp"])
                        nc.scalar.activation(o[:], acc[:], ACTF.Copy)
                        outt.append(o)
                    return outt

                qp = proj(wq_sb, qt, Bqp, "qp")
                kp = proj(wk_sb, kt, Bkp, "kp")

                # v projection: (t, f) tiles [t = S axis]
                vp = []
                for st in range(NST):
                    scols = slice(st * 128, st * 128 + STW)
                    acc = ps_a.tile([STW, E], F32, name=f"psv{st}", tag="ps_mm")
                    for eb in range(EB):
                        nc.tensor.matmul(acc[:], xth[eb][:, scols], wv_sb[eb][:],
                                         start=(eb == 0), stop=False)
                    nc.tensor.matmul(acc[:], ones_row[:1, :STW], biasrow[2][:1, :],
                                     start=False, stop=True)
                    o = epool.tile([STW, E], F16 if VDT else F32,
                                   name=f"vp_{st}", tag="vp", bufs=NST + 2)
                    nc.scalar.activation(o[:], acc[:], ACTF.Copy)
                    vp.append(o)

                # shift scores (s, t) -> negated lse rows
                lserow = []
                for st in range(NST):
                    scols = slice(st * 128, st * 128 + STW)
                    nmax_c = epool.tile([STW, H], F32, name=f"nmaxc{st}",
                                        tag="nmaxc", bufs=NST + 1)
                    den_c = epool.tile([STW, H], F32, name=f"denc{st}",
                                       tag="denc", bufs=NST + 1)
                    for h in range(H):
                        accs = ps_b.tile([STW, S], F32, name=f"pssh{st}{h}", tag="ps_sc")
                        nc.tensor.matmul(accs[:], qp[h][:, scols], kp[h][:],
                                         start=True, stop=True)
                        nc.vector.tensor_reduce(
                            nmax_c[:, h:h + 1], accs[:], axis=AX.X,
                            op=ALU.max, negate=True)
                        scr = epool.tile([STW, S], F16, name="escr", tag="escr")
                        nc.scalar.activation(
                            scr[:], accs[:], ACTF.Exp,
                            bias=nmax_c[:, h:h + 1], scale=1.0,
                            accum_out=den_c[:, h:h + 1])
                    ln_c = epool.tile([STW, H], F32, name=f"lnc{st}", tag="lnc",
                                      bufs=NST + 1)
                    nc.scalar.activation(ln_c[:], den_c[:], ACTF.Ln)
                    lse_c = epool.tile([STW, H], F32, name=f"lsec{st}", tag="lsec",
                                       bufs=NST + 1)
                    nc.vector.tensor_tensor(lse_c[:], nmax_c[:], ln_c[:],
                                            op=ALU.subtract)  # -(max) - ln(den)
                    # partition->free rearrange via DRAM bounce
                    bnc = dbpool.tile([STW, H], F32, name=f"lsebnc{st}", tag="lsebnc")
                    nc.sync.dma_start(bnc[:], lse_c[:])
                    lr = epool.tile([1, STW * H], F32, name=f"lserow{st}",
                                    tag="lserow", bufs=NST + 1)
                    nc.sync.dma_start(lr[:], bnc[:].rearrange("s h -> (s h)").unsqueeze(0))
                    lserow.append(lr)

                def hrow(rows, st, h):
                    # strided (1, STW) view: elements [h], [H+h], ...
                    return rows[st][:].rearrange("o (s h) -> o s h", h=H)[:, :, h]

                # scores^T - lse -> exp -> normalized attn^T (t, s), per head
                PT = []
                for h in range(H):
                    row = []
                    for tt in range(NST):
                        tcols = slice(tt * 128, tt * 128 + STW)
                        acc = ps_b.tile([STW, S], F32, name=f"psT{h}{tt}", tag="ps_sc")
                        nc.tensor.matmul(acc[:], kp[h][:, tcols], qp[h][:],
                                         start=True, stop=False)
                        for st in range(NST):
                            scols = slice(st * 128, st * 128 + STW)
                            nc.tensor.matmul(
                                acc[:, scols], ones_row[:1, :STW],
                                hrow(lserow, st, h),
                                start=False, stop=(st == NST - 1))
                        p = epool.tile([STW, S], F16 if VDT else F32,
                                       name=f"PT{h}{tt}", tag="PT", bufs=cfg.tune["PT"])
                        nc.scalar.activation(p[:], acc[:], ACTF.Exp)
                        row.append(p)
                    PT.append(row)

                # attn @ V -> o^T (hd, s) per head
                osc = []
                for h in range(H):
                    hr = slice(h * HD, (h + 1) * HD)
                    acc = ps_t.tile([HD, S], F32, name=f"pso{h}", tag="ps_oo")
                    for tt in range(NST):
                        nc.tensor.matmul(acc[:], vp[tt][:, hr], PT[h][tt][:],
                                         start=(tt == 0), stop=(tt == NST - 1))
                    o = epool.tile([HD, S], F16 if VDT else F32,
                                   name=f"osc{h}", tag="osc", bufs=H + 1)
                    nc.scalar.activation(o[:], acc[:], ACTF.Copy)
                    osc.append(o)

                # out projection: (g, s) tiles -> out[g, n, s]
                for gm in range(EB):
                    gr = slice(gm * 128, (gm + 1) * 128)
                    acc = ps_a.tile([128, S], F32, name=f"psout{gm}", tag="ps_mm")
                    for fm in range(EB):
                        nc.tensor.matmul(acc[:], wo_v[fm][:, gr], osc[fm][:],
                                         start=(fm == 0), stop=False)
                    nc.tensor.matmul(acc[:], biasrow[3][:1, gr], ones_row[:1, :S],
                                     start=False, stop=True)
                    o = epool.tile([128, S], F32, name=f"oo{gm}", tag="oo")
                    nc.scalar.activation(o[:], acc[:], ACTF.Copy)
                    nc.scalar.dma_start(out[gr, n, :], o[:])

            def emit_body():
                # ================= band =================
                for eb in range(EB if not cfg.skip_band else 0):
                    er = slice(eb * 128, (eb + 1) * 128)
                    for ss in range(NSS):
                        sr = slice(ss * SB, (ss + 1) * SB)
                        xb = bpool.tile([128, BW], F32, name="xb", tag="xb")
                        nc.sync.dma_start(xb[:], xband[er, sr, :])
                        xpb = bpool.tile([128, BW], F32, name="xpb", tag="xpb")
                        nc.sync.dma_start(xpb[:], xp[er, sr, :])

                        x3 = xb[:].rearrange("p (s l) -> p s l", l=CH)
                        xp3 = xpb[:].rearrange("p (s l) -> p s l", l=CH)

                        def prod(kind, src3, nm):
                            p = bpool.tile([128, BW], F32, name=nm, tag="prod",
                                           bufs=cfg.tune["prod"])
                            wb = wband_sb[kind][eb][:].unsqueeze(1) \
                                .broadcast_to([128, SB, CH])
                            nc.vector.tensor_tensor(
                                p[:].rearrange("p (s l) -> p s l", l=CH),
                                src3, wb, op=ALU.mult)
                            return p

                        def scan(p, nm):
                            o = spool.tile([128, BW], F32, name=nm, tag="scan",
                                           bufs=cfg.tune["scan"])
                            nc.vector.tensor_tensor_scan(
                                o[:], gate[:], p[:], 0.0,
                                op0=ALU.mult, op1=ALU.add)
                            return o

                        def assemble(I_fwd, I_sum_p, I_sum_own, qk, nm):
                            # out = x + E_fwd(shifted I_fwd) + (I_sum_p - I_sum_own)
                            t1 = bpool.tile([128, BW], F32, name=f"t1{nm}", tag="t1")
                            t13 = t1[:].rearrange("p (s l) -> p s l", l=CH)
                            I3 = I_fwd[:].rearrange("p (s l) -> p s l", l=CH)
                            nc.vector.tensor_tensor(
                                t13[:, :, 1:CH], x3[:, :, 1:CH], I3[:, :, 0:CH - 1],
                                op=ALU.add)
                            nc.vector.tensor_copy(t13[:, :, 0:1], x3[:, :, 0:1])
                            ts = bpool.tile([128, BW], F32, name=f"ts{nm}", tag="ts")
                            nc.vector.tensor_tensor(
                                ts[:], I_sum_p[:], I_sum_own[:], op=ALU.subtract)
                            o = bpool.tile([128, BW], F32, name=f"o{nm}", tag="qk")
                            nc.vector.tensor_tensor(o[:], t1[:], ts[:], op=ALU.add)
                            # free-dim permute (s,l)->(l,s) on GpSimd, then a
                            # contiguous-run store
                            o2 = bpool.tile([128, BW], F32, name=f"o2{nm}", tag="qk2")
                            nc.gpsimd.tensor_copy(
                                o2[:].rearrange("p (l s) -> p l s", s=SB),
                                o[:].rearrange("p (s l) -> p l s", l=CH))
                            dram = qdram if qk == "q" else kdram
                            nc.scalar.dma_start(
                                dram[er, 0:CH, sr],
                                o2[:].rearrange("p (l s) -> p l s", s=SB))

                        pa = prod(0, x3, "pa"); Ia = scan(pa, "Ia")
                        pc = prod(2, x3, "pc"); Ic = scan(pc, "Ic")
                        pp1 = prod(4, xp3, "pp1"); Ip1 = scan(pp1, "Ip1")
                        assemble(Ia, Ip1, Ic, "q", "q")
                        pb_ = prod(1, x3, "pb"); Ib = scan(pb_, "Ib")
                        pd = prod(3, x3, "pd"); Id = scan(pd, "Id")
                        pp2 = prod(5, xp3, "pp2"); Ip2 = scan(pp2, "Ip2")
                        assemble(Ib, Ip2, Id, "k", "k")

                        # totals -> tin[kind, e, s]
                        for kind, I in ((0, Ia), (1, Ib), (2, Ic), (3, Id)):
                            tv = I[:].rearrange("p (s l) -> p s l", l=CH)[:, :, CH - 1]
                            nc.sync.dma_start(tin[kind, er, sr], tv)

                # ================= totals exchange + B =================
                if not cfg.no_collective:
                    nc.gpsimd.collective_compute(
                        "AllGather", ALU.bypass,
                        replica_groups=[list(range(NC))],
                        ins=[tin[:, :, :]], outs=[tout[:, :, :]],
                    )
                # B_q/B_k per e-block: (128, S)
                Bq_eb, Bk_eb = [], []
                for eb in range(EB):
                    er = slice(eb * 128, (eb + 1) * 128)
                    for qk, kinds, dst in (("q", (0, 2), Bq_eb), ("k", (1, 3), Bk_eb)):
                        acc = cpool.tile([128, S], F32, name=f"B{qk}_{eb}")
                        nc.vector.memset(acc[:], 0.0)
                        for j in range(NC):
                            for ci, kind in enumerate(kinds):
                                tsl = bcpool.tile([128, S], F32, name="tsl", tag="tsl")
                                nc.sync.dma_start(tsl[:], tout[4 * j + kind, er, :])
                                nc.vector.scalar_tensor_tensor(
                                    acc[:], tsl[:],
                                    coef_sb[:, ci * NC + j:ci * NC + j + 1],
                                    acc[:], op0=ALU.mult, op1=ALU.add)
                        dst.append(acc)

                # B_proj (f-tiles) = W^T B + bias, kept in SBUF
                def bproj(w_sb, B_eb, bias_j, nm):
                    tiles = []
                    for fm in range(EB):
                        fr = slice(fm * 128, (fm + 1) * 128)
                        acc = ps_a.tile([128, S], F32, name=f"psB{nm}{fm}", tag="ps_mm")
                        for eb in range(EB):
                            nc.tensor.matmul(acc[:], w_sb[eb][:, fr], B_eb[eb][:],
                                             start=(eb == 0), stop=(eb == EB - 1))
                        o = cpool.tile([128, S], F32, name=f"B{nm}p_{fm}")
                        nc.vector.tensor_scalar_add(
                            o[:], acc[:],
                            biasc_sb[:, 4 * fm + bias_j:4 * fm + bias_j + 1])
                        tiles.append(o)
                    return tiles

                Bqp = bproj(wq_sb, Bq_eb, 0, "q")
                Bkp = bproj(wk_sb, Bk_eb, 1, "k")

                # ================= per-n attention =================
                NMAX = cfg.nmax if not cfg.skip_attn else 0
                for n0 in range(0, NMAX, NPAIR):
                    qt2, kt2, xt2 = [], [], []
                    for eb in range(EB):
                        er = slice(eb * 128, (eb + 1) * 128)
                        nsl = slice(n0, n0 + NPAIR)
                        t = dpool.tile([128, NPAIR * S], F32, name=f"qt{eb}",
                                       tag=f"qt{eb}")
                        nc.sync.dma_start(t[:], qdram[er, nsl, :])
                        qt2.append(t)
                        t = dpool.tile([128, NPAIR * S], F32, name=f"kt{eb}",
                                       tag=f"kt{eb}")
                        nc.sync.dma_start(t[:], kdram[er, nsl, :])
                        kt2.append(t)
                        t = dpool.tile([128, NPAIR * S], F32, name=f"xt{eb}",
                                       tag=f"xt{eb}")
                        nc.sync.dma_start(t[:], xattn[er, nsl, :])
                        xt2.append(t)
                    if VDT is not None:
                        xth2 = []
                        for eb in range(EB):
                            t = dpool.tile([128, NPAIR * S], F16, name=f"xth{eb}",
                                           tag=f"xth{eb}")
                            nc.vector.tensor_copy(t[:], xt2[eb][:])
                            xth2.append(t)
                    else:
                        xth2 = xt2

                    for j in range(NPAIR):
                        if n0 + j >= NMAX:
                            break
                        jsl = slice(j * S, (j + 1) * S)
                        emit_attn(n0 + j,
                                  [t[:, jsl] for t in qt2],
                                  [t[:, jsl] for t in kt2],
                                  [t[:, jsl] for t in xth2],
                                  Bqp, Bkp)

            for _rep in range(cfg.reps):
                emit_body()

    nc.finalize()
    return nc


# ============================================================
# host side
# ============================================================

def prep_inputs(cfg: Cfg, x, a, b, c, d, in_proj_w, in_proj_b, out_w, out_b):
    S, L, E, NC, CH, OFF = cfg.S, cfg.L, cfg.E, cfg.NC, cfg.CH, cfg.OFF
    f32 = np.float32
    x = np.asarray(x, f32)
    xg = np.ascontiguousarray(x.transpose(2, 0, 1))     # (E, S, L)
    hd = cfg.HD
    scl = 1.0 / math.sqrt(hd)
    vdt = np.float16 if cfg.v_dtype == "fp16" else f32
    wq = np.ascontiguousarray(in_proj_w[:E].T * scl).astype(f32)
    wk = np.ascontiguousarray(in_proj_w[E:2 * E].T).astype(f32)
    wv = np.ascontiguousarray(in_proj_w[2 * E:].T).astype(vdt)
    wo = np.ascontiguousarray(out_w.T).astype(vdt)
    bq = in_proj_b[:E] * scl
    bk = in_proj_b[E:2 * E]
    bv = in_proj_b[2 * E:]
    bo = out_b
    biasr = np.stack([bq, bk, bv, bo]).astype(f32)
    biasc = np.ascontiguousarray(biasr.T).astype(f32)
    ident = np.eye(128, dtype=f32)

    gate = np.ones((128, cfg.SB * CH), f32)
    gate[:, ::CH] = 0.0

    in_maps = []
    for k in range(NC):
        chk = slice(CH * k, CH * (k + 1))
        xbandc = np.ascontiguousarray(xg[:, :, chk])
        xattnc = np.ascontiguousarray(xg[:, :, chk].transpose(0, 2, 1))
        if k >= OFF:
            pf = slice(CH * (k - OFF), CH * (k - OFF + 1))
            xpc = np.ascontiguousarray(xg[:, :, pf])
            w1 = -a[pf].astype(f32)
            w2 = -b[pf].astype(f32)
        else:
            st = CH * (k + OFF) - 1
            xpc = np.zeros((E, S, CH), f32)
            xpc[:, :, 1:] = xg[:, :, st + 1:st + CH]
            w1 = np.zeros((CH, E), f32)
            w1[1:] = c[st + 1:st + CH]
            w2 = np.zeros((CH, E), f32)
            w2[1:] = d[st + 1:st + CH]
        wbandc = np.ascontiguousarray(
            np.stack([a[chk], b[chk], c[chk], d[chk], w1, w2])
            .transpose(0, 2, 1)).astype(f32)          # (6, E, CH)
        coefA = np.zeros(NC, f32)
        coefA[max(0, k - OFF):k] = 1.0
        coefC = np.zeros(NC, f32)
        coefC[k:min(k + OFF - 1, NC - 1) + 1] = 1.0
        coefv = np.broadcast_to(
            np.concatenate([coefA, coefC])[None, :], (128, 2 * NC)).copy()
        in_maps.append(dict(
            xband=xbandc, xattn=xattnc, xp=xpc,
            wband=wbandc, gate_in=gate, coef=coefv,
            wq=wq, wk=wk, wv=wv, wo=wo, biasr=biasr, biasc=biasc,
            ident_in=ident,
        ))
    return in_maps


_CACHE = {}


def run(cfg: Cfg, inputs, core_ids=None, **kw):
    key = cfg.key()
    if key not in _CACHE:
        _CACHE[key] = build_nc(cfg)
    nc = _CACHE[key]
    in_maps = prep_inputs(
        cfg, inputs["x"], inputs["a"], inputs["b"], inputs["c"], inputs["d"],
        inputs["in_proj_w"], inputs["in_proj_b"], inputs["out_w"], inputs["out_b"])
    res = run_bass_kernel_spmd(nc, in_maps, core_ids or list(range(cfg.NC)), **kw)
    S, L, E, CH = cfg.S, cfg.L, cfg.E, cfg.CH
    full = np.empty((S, L, E), np.float32)
    for k in range(cfg.NC):
        # out is (E, CH, S)
        full[:, CH * k:CH * (k + 1), :] = res.results[k]["out"].transpose(2, 1, 0)
    return full, res


def kernel(**inputs) -> np.ndarray:
    assert int(inputs["n1"]) == 256 and int(inputs["n2"]) == 256
    cfg = Cfg()
    out, _ = run(cfg, inputs)
    return out



# revision 15
# speedup vs baseline: 4.5598x; 4.5598x over previous
"""Trainium2 Bass kernel for nn_MultiHeadSSAN: banded Q/K (prefix-sum windows
along feature_len) + multi-head self-attention, sharded over the feature_len
(L) axis across 8 NeuronCores.

v2 design (fp16 end-to-end data path, fp32 PSUM/scan accumulation):

  Band:  per (eb, ss) tile [128e, SB*CH] fp16, s-major l-inner.
         Products x*w split across GpSimd (4) and DVE (2); the fwd products
         are pre-shifted one step along l so a single inclusive
         tensor_tensor_scan yields the exclusive prefix directly (scan state
         is fp32 internally).  Assemble is 3 contiguous fp16 adds; the (s,l)
         -> (l,s) free-dim permute runs on the Scalar engine; stores go out
         as fp16 with 128B runs split across the Scalar and Sync queues.
         Chunk totals are extracted with small strided DVE copies and
         AllGathered per-eb (4 small collectives) so communication overlaps
         the remaining band compute.
  MHA:   per n: all matmuls fp16 (1 cyc/row, FWL weight loads).  Boundary
         constants fold into q/k via the PSUM-evacuation adds (no identity
         matmuls).  Softmax: (s,t) scores give -max (DVE reduce) and den
         (exp accum), lse = -(max+ln den) is split hi/lo fp16, PE-transposed
         to rows, and folded into the transposed (t,s) scores via rank-1
         PSUM accumulates, so the second exp emits normalized attn^T
         directly.  Output is stored fp16 and upcast on host.
"""
import math
import numpy as np

import concourse.bass as bass
import concourse.bacc as bacc
import concourse.mybir as mybir
import concourse.tile as tile
from concourse.bass_utils import run_bass_kernel_spmd

F32 = mybir.dt.float32
F16 = mybir.dt.float16
ALU = mybir.AluOpType
ACTF = mybir.ActivationFunctionType
AX = mybir.AxisListType


class Cfg:
    def __init__(self, S=256, L=512, E=512, H=4, NC=8, OFF=4, SB=64,
                 no_collective=False, reps=1,
                 skip_band=False, skip_attn=False, nmax=None, tune=None):
        self.S, self.L, self.E, self.H, self.NC = S, L, E, H, NC
        self.CH = L // NC              # L-chunk per core
        self.OFF = OFF                 # partner offset = n1 // CH
        assert OFF * 2 >= NC, "single-partner scheme needs OFF >= NC/2"
        self.n1 = self.n2 = OFF * self.CH
        self.HD = E // H
        assert self.HD == 128 and E % 128 == 0
        self.EB = E // 128             # e partition blocks
        self.SB = SB                   # band s-sub size
        assert S % SB == 0
        self.NSS = S // SB
        self.NST = (S + 127) // 128    # s tiles of <=128 in attention
        self.STW = min(128, S)
        assert self.STW == 128 and self.NST == 2
        self.no_collective = no_collective
        self.reps = reps
        self.skip_band = skip_band
        self.skip_attn = skip_attn
        self.nmax = nmax if nmax is not None else self.CH
        self.tune = dict(ps_a=3, ps_b=2, ps_t=2, dpool=2, scan=5, qkp=6, PT=10,
                         vp=4, osc=3, oo=3)
        if tune: self.tune.update(tune)

    def key(self):
        return (self.S, self.L, self.E, self.H, self.NC, self.OFF, self.SB,
                self.no_collective, self.reps,
                self.skip_band, self.skip_attn, self.nmax,
                tuple(sorted(self.tune.items())))


def build_nc(cfg: Cfg) -> bass.Bass:
    S, L, E, H, NC = cfg.S, cfg.L, cfg.E, cfg.H, cfg.NC
    CH, EB, SB, HD = cfg.CH, cfg.EB, cfg.SB, cfg.HD
    NSS = cfg.NSS
    BW = SB * CH                       # band tile free width
    NST, STW = cfg.NST, cfg.STW
    NPAIR = 2

    nc = bacc.Bacc(None)
    # ---- parameters
    xband = nc.declare_dram_parameter("xband", [E, S, CH], F16, isOutput=False)
    xattn = nc.declare_dram_parameter("xattn", [E, CH, S], F16, isOutput=False)
    xp = nc.declare_dram_parameter("xp", [E, S, CH], F16, isOutput=False)
    wband = nc.declare_dram_parameter("wband", [6, E, CH], F16, isOutput=False)
    gate_in = nc.declare_dram_parameter("gate_in", [128, BW], F16, isOutput=False)
    coef = nc.declare_dram_parameter("coef", [128, 2 * NC], F32, isOutput=False)
    wq = nc.declare_dram_parameter("wq", [E, E], F16, isOutput=False)
    wk = nc.declare_dram_parameter("wk", [E, E], F16, isOutput=False)
    wv = nc.declare_dram_parameter("wv", [E, E], F16, isOutput=False)
    wo = nc.declare_dram_parameter("wo", [E, E], F16, isOutput=False)
    biasv = nc.declare_dram_parameter("biasv", [1, E], F16, isOutput=False)
    wlast = nc.declare_dram_parameter("wlast", [E, 2], F32, isOutput=False)
    biasc = nc.declare_dram_parameter("biasc", [E, 4], F32, isOutput=False)
    ident_in = nc.declare_dram_parameter("ident_in", [128, 128], F16, isOutput=False)
    out = nc.declare_dram_parameter("out", [E, CH, S], F16, isOutput=True)

    # ---- internal DRAM
    qdram = nc.dram_tensor("qdram", [E, CH, S], F16)
    kdram = nc.dram_tensor("kdram", [E, CH, S], F16)
    tin = [nc.dram_tensor(f"tin{eb}", [4, 128, S], F16) for eb in range(EB)]
    tout = [nc.dram_tensor(f"tout{eb}", [4 * NC, 128, S], F16,
                           addr_space="Shared") for eb in range(EB)]

    with tile.TileContext(nc) as tc:
        with (
            tc.tile_pool(name="const", bufs=1) as cpool,
            tc.tile_pool(name="ps_a", bufs=cfg.tune["ps_a"], space="PSUM") as ps_a,
            tc.tile_pool(name="ps_b", bufs=cfg.tune["ps_b"], space="PSUM") as ps_b,
            tc.tile_pool(name="ps_t", bufs=cfg.tune["ps_t"], space="PSUM") as ps_t,
        ):
            # ================= constants =================
            gate = cpool.tile([128, BW], F16, name="gate")
            nc.sync.dma_start(gate[:], gate_in[:, :])
            ident = cpool.tile([128, 128], F16, name="ident")
            nc.sync.dma_start(ident[:], ident_in[:, :])
            coef_sb = cpool.tile([128, 2 * NC], F32, name="coef_sb")
            nc.sync.dma_start(coef_sb[:], coef[:, :])
            biasv_sb = cpool.tile([1, E], F16, name="biasv_sb")
            nc.sync.dma_start(biasv_sb[:], biasv[:, :])
            wlast_sb = []
            for eb in range(EB):
                t = cpool.tile([128, 2], F32, name=f"wlast_{eb}")
                nc.sync.dma_start(t[:], wlast[eb * 128:(eb + 1) * 128, :])
                wlast_sb.append(t)
            biasc_sb = cpool.tile([128, 4 * EB], F32, name="biasc_sb")
            for eb in range(EB):
                nc.sync.dma_start(biasc_sb[:, 4 * eb:4 * (eb + 1)],
                                  biasc[eb * 128:(eb + 1) * 128, :])
            ones_row = cpool.tile([1, max(S, 128)], F16, name="ones_row")
            nc.vector.memset(ones_row[:], 1.0)

            wband_sb = []
            for kind in range(6):
                row = []
                for eb in range(EB):
                    t = cpool.tile([128, CH], F16, name=f"wband_{kind}_{eb}")
                    nc.sync.dma_start(t[:], wband[kind, eb * 128:(eb + 1) * 128, :])
                    row.append(t)
                wband_sb.append(row)

            def load_w(dram, nm):
                tiles = []
                for eb in range(EB):
                    t = cpool.tile([128, E], F16, name=f"{nm}_{eb}")
                    nc.sync.dma_start(t[:], dram[eb * 128:(eb + 1) * 128, :])
                    tiles.append(t)
                return tiles

            wq_sb = load_w(wq, "wq")
            wk_sb = load_w(wk, "wk")
            wv_sb = load_w(wv, "wv")
            wo_sb = load_w(wo, "wo")

            # totals accumulators (per eb, 4 kinds side by side on free axis)
            tin_sb = [cpool.tile([128, 4 * S], F16, name=f"tin_sb{eb}")
                      for eb in range(EB)]

            def emit_band():
                with (
                    tc.tile_pool(name="bin", bufs=2) as binp,
                    tc.tile_pool(name="prod", bufs=2) as ppool,
                    tc.tile_pool(name="scan", bufs=cfg.tune["scan"]) as spool,
                    tc.tile_pool(name="asm", bufs=2) as apool,
                ):
                    for eb in range(EB):
                        er = slice(eb * 128, (eb + 1) * 128)
                        for ss in range(NSS):
                            sr = slice(ss * SB, (ss + 1) * SB)
                            xb = binp.tile([128, BW], F16, name="xb", tag="xb")
                            nc.sync.dma_start(xb[:], xband[er, sr, :])
                            xpb = binp.tile([128, BW], F16, name="xpb", tag="xpb")
                            nc.gpsimd.dma_start(xpb[:], xp[er, sr, :])
                            x3 = xb[:].rearrange("p (s l) -> p s l", l=CH)
                            xp3 = xpb[:].rearrange("p (s l) -> p s l", l=CH)

                            def prod(kind, src3, nm, eng, shifted):
                                p = ppool.tile([128, BW], F16, name=nm,
                                               tag=f"prod_{eng}")
                                p3 = p[:].rearrange("p (s l) -> p s l", l=CH)
                                e = nc.gpsimd if eng == "g" else nc.vector
                                w = wband_sb[kind][eb]
                                if shifted:
                                    e.memset(p3[:, :, 0:1], 0.0)
                                    wb = w[:, 0:CH - 1].unsqueeze(1) \
                                        .broadcast_to([128, SB, CH - 1])
                                    e.tensor_tensor(p3[:, :, 1:CH],
                                                    src3[:, :, 0:CH - 1], wb,
                                                    op=ALU.mult)
                                else:
                                    wb = w[:].unsqueeze(1) \
                                        .broadcast_to([128, SB, CH])
                                    e.tensor_tensor(p3, src3, wb, op=ALU.mult)
                                return p

                            def scan(p, nm):
                                o = spool.tile([128, BW], F16, name=nm, tag="scan")
                                nc.vector.tensor_tensor_scan(
                                    o[:], gate[:], p[:], 0.0,
                                    op0=ALU.mult, op1=ALU.add)
                                return o

                            def half(qk, kf, ks, kp_, dram, store_eng):
                                # kf: fwd kind (shifted->exclusive), ks: bwd
                                # kind (inclusive), kp_: partner kind
                                pf = prod(kf, x3, "pf", "g", True)
                                ps_ = prod(ks, x3, "ps", "g", False)
                                pp = prod(kp_, xp3, "pp", "v", False)
                                If = scan(pf, "If")
                                Is = scan(ps_, "Is")
                                Ip = scan(pp, "Ip")
                                If3 = If[:].rearrange("p (s l) -> p s l", l=CH)
                                Is3 = Is[:].rearrange("p (s l) -> p s l", l=CH)
                                # totals: fwd kind needs the missing last
                                # product added back; bwd kind is inclusive
                                koff = kf * S
                                tmp = apool.tile([128, SB], F16, name="tfix",
                                                 tag="tfix")
                                nc.vector.tensor_scalar_mul(
                                    tmp[:], x3[:, :, CH - 1],
                                    wlast_sb[eb][:, kf:kf + 1])
                                nc.vector.tensor_tensor(
                                    tin_sb[eb][:, koff + ss * SB:koff + (ss + 1) * SB],
                                    If3[:, :, CH - 1], tmp[:], op=ALU.add)
                                nc.vector.tensor_copy(
                                    tin_sb[eb][:, ks * S + ss * SB:ks * S + (ss + 1) * SB],
                                    Is3[:, :, CH - 1])
                                # assemble: out = x + If_ex + (Ip - Is)
                                ts = apool.tile([128, BW], F16, name=f"ts{qk}",
                                                tag="ts")
                                nc.vector.tensor_tensor(ts[:], Ip[:], Is[:],
                                                        op=ALU.subtract)
                                t1 = apool.tile([128, BW], F16, name=f"t1{qk}",
                                                tag="t1")
                                nc.vector.tensor_tensor(t1[:], xb[:], ts[:],
                                                        op=ALU.add)
                                t2 = apool.tile([128, BW], F16, name=f"t2{qk}",
                                                tag="ts")
                                nc.vector.tensor_tensor(t2[:], t1[:], If[:],
                                                        op=ALU.add)
                                # free-dim permute (s,l)->(l,s) on Scalar
                                o2 = apool.tile([128, BW], F16, name=f"o2{qk}",
                                                tag="o2")
                                nc.scalar.copy(
                                    o2[:].rearrange("p (l s) -> p l s", s=SB),
                                    t2[:].rearrange("p (s l) -> p l s", l=CH))
                                store_eng.dma_start(
                                    dram[er, 0:CH, sr],
                                    o2[:].rearrange("p (l s) -> p l s", s=SB))

                            half("q", 0, 2, 4, qdram, nc.scalar)
                            half("k", 1, 3, 5, kdram, nc.sync)

                        # per-eb totals store + collective
                        for kind in range(4):
                            nc.sync.dma_start(
                                tin[eb][kind, :, :],
                                tin_sb[eb][:, kind * S:(kind + 1) * S])
                        if not cfg.no_collective:
                            nc.gpsimd.collective_compute(
                                "AllGather", ALU.bypass,
                                replica_groups=[list(range(NC))],
                                ins=[tin[eb][:, :, :]], outs=[tout[eb][:, :, :]],
                            )

            # ================= B-terms =================
            def emit_b():
                with tc.tile_pool(name="bterm", bufs=2) as btp:
                    Bqp, Bkp = [], []  # fp16 [128, S] per fm, proj-space
                    Bq_eb, Bk_eb = [], []
                    for eb in range(EB):
                        for qk, kinds, dst in (("q", (0, 2), Bq_eb),
                                               ("k", (1, 3), Bk_eb)):
                            acc = cpool.tile([128, S], F32, name=f"B{qk}_{eb}")
                            nc.vector.memset(acc[:], 0.0)
                            for j in range(NC):
                                for ci, kind in enumerate(kinds):
                                    tsl = btp.tile([128, S], F16, name="tsl",
                                                   tag="tsl", bufs=3)
                                    nc.sync.dma_start(
                                        tsl[:], tout[eb][4 * j + kind, :, :])
                                    nc.vector.scalar_tensor_tensor(
                                        acc[:], tsl[:],
                                        coef_sb[:, ci * NC + j:ci * NC + j + 1],
                                        acc[:], op0=ALU.mult, op1=ALU.add)
                            h = cpool.tile([128, S], F16, name=f"B16{qk}_{eb}")
                            nc.vector.tensor_copy(h[:], acc[:])
                            dst.append(h)
                    for qk, w_sb, B_eb, bj, dst in (
                            ("q", wq_sb, Bq_eb, 0, Bqp),
                            ("k", wk_sb, Bk_eb, 1, Bkp)):
                        for fm in range(EB):
                            fr = slice(fm * 128, (fm + 1) * 128)
                            acc = ps_a.tile([128, S], F32, name=f"psB{qk}{fm}",
                                            tag="ps_mm")
                            for eb in range(EB):
                                nc.tensor.matmul(acc[:], w_sb[eb][:, fr],
                                                 B_eb[eb][:],
                                                 start=(eb == 0), stop=(eb == EB - 1))
                            o = cpool.tile([128, S], F16, name=f"B{qk}p_{fm}")
                            nc.vector.tensor_scalar_add(
                                o[:], acc[:],
                                biasc_sb[:, 4 * fm + bj:4 * fm + bj + 1])
                            dst.append(o)
                    return Bqp, Bkp

            # ================= attention =================
            def emit_attn(n, qt, kt, xt, Bqp, Bkp, apool):
                T = cfg.tune

                # q/k projections: fm-PAIR tiles [128, 2*S] (one PSUM bank)
                def proj2(w_sb, src, Bp, nm):
                    outt = []
                    for fp_ in range(EB // 2):
                        acc = ps_a.tile([128, 2 * S], F32, name=f"ps{nm}{fp_}",
                                        tag="ps_mm")
                        for half_ in range(2):
                            fm = fp_ * 2 + half_
                            fr = slice(fm * 128, (fm + 1) * 128)
                            hs = slice(half_ * S, (half_ + 1) * S)
                            for eb in range(EB):
                                nc.tensor.matmul(acc[:, hs], w_sb[eb][:, fr],
                                                 src[eb],
                                                 start=(eb == 0), stop=(eb == EB - 1))
                        o = apool.tile([128, 2 * S], F16, name=f"{nm}_{fp_}",
                                       tag="qkp", bufs=T["qkp"])
                        for half_ in range(2):
                            fm = fp_ * 2 + half_
                            hs = slice(half_ * S, (half_ + 1) * S)
                            nc.vector.tensor_tensor(o[:, hs], acc[:, hs],
                                                    Bp[fm][:], op=ALU.add)
                        outt.append(o)
                    return outt

                qp2 = proj2(wq_sb, qt, Bqp, "qp")
                kp2 = proj2(wk_sb, kt, Bkp, "kp")

                def hview(p2, h):
                    # head h lives in pair tile h//2, half h%2
                    return p2[h // 2][:, (h % 2) * S:(h % 2 + 1) * S]

                # v projection: (s, e) tiles
                vp = []
                for st in range(NST):
                    scols = slice(st * 128, st * 128 + STW)
                    acc = ps_a.tile([STW, E], F32, name=f"psv{st}", tag="ps_mm")
                    for eb in range(EB):
                        nc.tensor.matmul(acc[:], xt[eb][:, scols], wv_sb[eb][:],
                                         start=(eb == 0), stop=False)
                    nc.tensor.matmul(acc[:], ones_row[:1, :STW], biasv_sb[:1, :],
                                     start=False, stop=True)
                    o = apool.tile([STW, E], F16, name=f"vp_{st}", tag="vp",
                                   bufs=T["vp"])
                    nc.vector.tensor_copy(o[:], acc[:])
                    vp.append(o)

                # pass 1: (s,t) scores -> -max, den -> -lse (hi/lo fp16)
                nmax_c = apool.tile([STW, 2 * H], F32, name="nmaxc",
                                    tag="nmaxc", bufs=2)
                den_c = apool.tile([STW, 2 * H], F32, name="denc", tag="denc",
                                   bufs=2)
                escr = apool.tile([STW, S], F16, name="escr", tag="escr", bufs=2)
                for st in range(NST):
                    scols = slice(st * 128, st * 128 + STW)
                    for h in range(H):
                        c = h * NST + st
                        accs = ps_b.tile([STW, S], F32, name=f"pssh{st}{h}",
                                         tag="ps_sc")
                        nc.tensor.matmul(accs[:], hview(qp2, h)[:, scols],
                                         hview(kp2, h), start=True, stop=True)
                        nc.vector.tensor_reduce(
                            nmax_c[:, c:c + 1], accs[:], axis=AX.X,
                            op=ALU.max, negate=True)
                        nc.scalar.activation(
                            escr[:], accs[:], ACTF.Exp,
                            bias=nmax_c[:, c:c + 1], scale=1.0,
                            accum_out=den_c[:, c:c + 1])
                ln_c = apool.tile([STW, 2 * H], F32, name="lnc", tag="lnc", bufs=2)
                nc.scalar.activation(ln_c[:], den_c[:], ACTF.Ln)
                lse32 = apool.tile([STW, 2 * H], F32, name="lse32", tag="lse32",
                                   bufs=2)
                nc.vector.tensor_tensor(lse32[:], nmax_c[:], ln_c[:],
                                        op=ALU.subtract)  # -(max+ln den)
                pk = apool.tile([STW, 4 * H], F16, name="lsepack", tag="lsepack",
                                bufs=2)
                nc.vector.tensor_copy(pk[:, 0:2 * H], lse32[:])
                resid = apool.tile([STW, 2 * H], F32, name="resid", tag="resid",
                                   bufs=2)
                nc.vector.tensor_tensor(resid[:], lse32[:], pk[:, 0:2 * H],
                                        op=ALU.subtract)
                nc.vector.tensor_copy(pk[:, 2 * H:4 * H], resid[:])
                # partition->free flatten: lseflat[0, s*16+r] = pk[s, r]
                lseflat = apool.tile([1, STW * 4 * H], F16, name="lseflat",
                                     tag="lseflat", bufs=3)
                nc.gpsimd.dma_start(
                    lseflat[:].rearrange("o (s r) -> o s r", r=4 * H),
                    pk[:])

                # pass 2: scores^T + rank-1(-lse) -> exp -> attn^T (t,s)
                PT = []
                for h in range(H):
                    row = []
                    for tt in range(NST):
                        tcols = slice(tt * 128, tt * 128 + STW)
                        acc = ps_b.tile([STW, S], F32, name=f"psT{h}{tt}",
                                        tag="ps_sc")
                        nc.tensor.matmul(acc[:], hview(kp2, h)[:, tcols],
                                         hview(qp2, h), start=True, stop=False)
                        lse3 = lseflat[:].rearrange("o (s r) -> o s r", r=4 * H)
                        for st in range(NST):
                            scols = slice(st * 128, st * 128 + STW)
                            c = h * NST + st
                            for part in range(2):
                                last = (st == NST - 1) and (part == 1)
                                nc.tensor.matmul(
                                    acc[:, scols], ones_row[:1, :STW],
                                    lse3[:, :, part * 2 * H + c],
                                    start=False, stop=last)
                        p = apool.tile([STW, S], F16, name=f"PT{h}{tt}",
                                       tag="PT", bufs=T["PT"])
                        nc.scalar.activation(p[:], acc[:], ACTF.Exp)
                        row.append(p)
                    PT.append(row)

                # attn @ V -> o^T (hd, s), h-PAIRS in one PSUM bank
                osc = []
                for hp in range(H // 2):
                    acc = ps_t.tile([HD, 2 * S], F32, name=f"pso{hp}", tag="ps_oo")
                    for half_ in range(2):
                        h = hp * 2 + half_
                        hr = slice(h * HD, (h + 1) * HD)
                        hs = slice(half_ * S, (half_ + 1) * S)
                        for tt in range(NST):
                            nc.tensor.matmul(acc[:, hs], vp[tt][:, hr],
                                             PT[h][tt][:],
                                             start=(tt == 0), stop=(tt == NST - 1))
                    o = apool.tile([HD, 2 * S], F16, name=f"osc{hp}", tag="osc",
                                   bufs=T["osc"])
                    nc.vector.tensor_copy(o[:], acc[:])
                    osc.append(o)

                def oview(f):
                    return osc[f // 2][:, (f % 2) * S:(f % 2 + 1) * S]

                # out projection -> out[g, n, s]
                for gp in range(EB // 2):
                    acc = ps_a.tile([128, 2 * S], F32, name=f"psout{gp}",
                                    tag="ps_mm")
                    for half_ in range(2):
                        gm = gp * 2 + half_
                        gr = slice(gm * 128, (gm + 1) * 128)
                        hs = slice(half_ * S, (half_ + 1) * S)
                        for fm in range(EB):
                            nc.tensor.matmul(acc[:, hs], wo_sb[fm][:, gr],
                                             oview(fm), start=(fm == 0),
                                             stop=(fm == EB - 1))
                    o = apool.tile([128, 2 * S], F16, name=f"oo{gp}", tag="oo",
                                   bufs=T["oo"])
                    for half_ in range(2):
                        gm = gp * 2 + half_
                        hs = slice(half_ * S, (half_ + 1) * S)
                        nc.vector.tensor_scalar_add(
                            o[:, hs], acc[:, hs],
                            biasc_sb[:, 4 * gm + 3:4 * gm + 4])
                    for half_ in range(2):
                        gm = gp * 2 + half_
                        gr = slice(gm * 128, (gm + 1) * 128)
                        hs = slice(half_ * S, (half_ + 1) * S)
                        nc.gpsimd.dma_start(out[gr, n, :], o[:, hs])

            def emit_attn_all(Bqp, Bkp):
                with (
                    tc.tile_pool(name="dpool", bufs=cfg.tune["dpool"]) as dpool,
                    tc.tile_pool(name="attn", bufs=2) as apool,
                ):
                    NMAX = cfg.nmax if not cfg.skip_attn else 0
                    for n0 in range(0, NMAX, NPAIR):
                        qt2, kt2, xt2 = [], [], []
                        nsl = slice(n0, n0 + NPAIR)
                        for eb in range(EB):
                            er = slice(eb * 128, (eb + 1) * 128)
                            t = dpool.tile([128, NPAIR * S], F16, name=f"qt{eb}",
                                           tag=f"qt{eb}")
                            nc.sync.dma_start(t[:], qdram[er, nsl, :])
                            qt2.append(t)
                            t = dpool.tile([128, NPAIR * S], F16, name=f"kt{eb}",
                                           tag=f"kt{eb}")
                            nc.sync.dma_start(t[:], kdram[er, nsl, :])
                            kt2.append(t)
                            t = dpool.tile([128, NPAIR * S], F16, name=f"xt{eb}",
                                           tag=f"xt{eb}")
                            nc.sync.dma_start(t[:], xattn[er, nsl, :])
                            xt2.append(t)
                        for j in range(NPAIR):
                            if n0 + j >= NMAX:
                                break
                            jsl = slice(j * S, (j + 1) * S)
                            emit_attn(n0 + j,
                                      [t[:, jsl] for t in qt2],
                                      [t[:, jsl] for t in kt2],
                                      [t[:, jsl] for t in xt2],
                                      Bqp, Bkp, apool)

            for _rep in range(cfg.reps):
                if not cfg.skip_band:
                    emit_band()
                Bqp, Bkp = emit_b()
                emit_attn_all(Bqp, Bkp)

    nc.finalize()
    return nc


# ============================================================
# host side
# ============================================================

def prep_inputs(cfg: Cfg, x, a, b, c, d, in_proj_w, in_proj_b, out_w, out_b):
    S, L, E, NC, CH, OFF = cfg.S, cfg.L, cfg.E, cfg.NC, cfg.CH, cfg.OFF
    f32, f16 = np.float32, np.float16
    x = np.asarray(x, f32)
    xg = np.ascontiguousarray(x.transpose(2, 0, 1))     # (E, S, L)
    hd = cfg.HD
    scl = 1.0 / math.sqrt(hd)
    wq = np.ascontiguousarray(in_proj_w[:E].T * scl).astype(f16)
    wk = np.ascontiguousarray(in_proj_w[E:2 * E].T).astype(f16)
    wv = np.ascontiguousarray(in_proj_w[2 * E:].T).astype(f16)
    wo = np.ascontiguousarray(out_w.T).astype(f16)
    bq = in_proj_b[:E] * scl
    bk = in_proj_b[E:2 * E]
    bv = in_proj_b[2 * E:]
    bo = out_b
    biasv = np.asarray(bv, f16).reshape(1, E)
    # last-column fwd weights (per core below)
    biasc = np.ascontiguousarray(
        np.stack([bq, bk, bv, bo]).astype(f32).T)       # (E, 4)
    ident = np.eye(128, dtype=f16)

    gate = np.ones((128, cfg.SB * CH), f16)
    gate[:, ::CH] = 0.0

    in_maps = []
    for k in range(NC):
        chk = slice(CH * k, CH * (k + 1))
        xbandc = np.ascontiguousarray(xg[:, :, chk]).astype(f16)
        xattnc = np.ascontiguousarray(
            xg[:, :, chk].transpose(0, 2, 1)).astype(f16)
        if k >= OFF:
            pf = slice(CH * (k - OFF), CH * (k - OFF + 1))
            xpc = np.ascontiguousarray(xg[:, :, pf]).astype(f16)
            w1 = -a[pf].astype(f32)
            w2 = -b[pf].astype(f32)
        else:
            st = CH * (k + OFF) - 1
            xpc = np.zeros((E, S, CH), f16)
            xpc[:, :, 1:] = xg[:, :, st + 1:st + CH]
            w1 = np.zeros((CH, E), f32)
            w1[1:] = c[st + 1:st + CH]
            w2 = np.zeros((CH, E), f32)
            w2[1:] = d[st + 1:st + CH]
        wbandc = np.ascontiguousarray(
            np.stack([a[chk], b[chk], c[chk], d[chk], w1, w2])
            .transpose(0, 2, 1)).astype(f16)            # (6, E, CH)
        coefA = np.zeros(NC, f32)
        coefA[max(0, k - OFF):k] = 1.0
        coefC = np.zeros(NC, f32)
        coefC[k:min(k + OFF - 1, NC - 1) + 1] = 1.0
        coefv = np.broadcast_to(
            np.concatenate([coefA, coefC])[None, :], (128, 2 * NC)).copy()
        wlastc = np.ascontiguousarray(
            np.stack([a[chk][CH - 1], b[chk][CH - 1]], axis=1)).astype(f32)
        in_maps.append(dict(
            xband=xbandc, xattn=xattnc, xp=xpc,
            wband=wbandc, gate_in=gate, coef=coefv, wlast=wlastc,
            wq=wq, wk=wk, wv=wv, wo=wo, biasv=biasv, biasc=biasc,
            ident_in=ident,
        ))
    return in_maps


_CACHE = {}


def run(cfg: Cfg, inputs, core_ids=None, **kw):
    key = cfg.key()
    if key not in _CACHE:
        _CACHE[key] = build_nc(cfg)
    nc = _CACHE[key]
    in_maps = prep_inputs(
        cfg, inputs["x"], inputs["a"], inputs["b"], inputs["c"], inputs["d"],
        inputs["in_proj_w"], inputs["in_proj_b"], inputs["out_w"], inputs["out_b"])
    res = run_bass_kernel_spmd(nc, in_maps, core_ids or list(range(cfg.NC)), **kw)
    S, L, E, CH = cfg.S, cfg.L, cfg.E, cfg.CH
    full = np.empty((S, L, E), np.float32)
    for k in range(cfg.NC):
        # out is (E, CH, S) fp16
        full[:, CH * k:CH * (k + 1), :] = \
            res.results[k]["out"].astype(np.float32).transpose(2, 1, 0)
    return full, res


def kernel(**inputs) -> np.ndarray:
    assert int(inputs["n1"]) == 256 and int(inputs["n2"]) == 256
    cfg = Cfg()
    out, _ = run(cfg, inputs)
    return out


# revision 23
# speedup vs baseline: 5.1125x; 1.1212x over previous
"""Trainium2 Bass kernel for nn_MultiHeadSSAN: banded Q/K (prefix-sum windows
along feature_len) + multi-head self-attention, sharded over the feature_len
(L) axis across 8 NeuronCores.

v2 design (fp16 end-to-end data path, fp32 PSUM/scan accumulation):

  Band:  per (eb, ss) tile [128e, SB*CH] fp16, s-major l-inner.
         Products x*w split across GpSimd (4) and DVE (2); the fwd products
         are pre-shifted one step along l so a single inclusive
         tensor_tensor_scan yields the exclusive prefix directly (scan state
         is fp32 internally).  Assemble is 3 contiguous fp16 adds; the (s,l)
         -> (l,s) free-dim permute runs on the Scalar engine; stores go out
         as fp16 with 128B runs split across the Scalar and Sync queues.
         Chunk totals are extracted with small strided DVE copies and
         AllGathered per-eb (4 small collectives) so communication overlaps
         the remaining band compute.
  MHA:   per n: all matmuls fp16 (1 cyc/row, FWL weight loads).  Boundary
         constants fold into q/k via the PSUM-evacuation adds (no identity
         matmuls).  Softmax: (s,t) scores give -max (DVE reduce) and den
         (exp accum), lse = -(max+ln den) is split hi/lo fp16, PE-transposed
         to rows, and folded into the transposed (t,s) scores via rank-1
         PSUM accumulates, so the second exp emits normalized attn^T
         directly.  Output is stored fp16 and upcast on host.
"""
import math
import numpy as np

import concourse.bass as bass
import concourse.bacc as bacc
import concourse.mybir as mybir
import concourse.tile as tile
from concourse.bass_utils import run_bass_kernel_spmd

F32 = mybir.dt.float32
F16 = mybir.dt.float16
ALU = mybir.AluOpType
ACTF = mybir.ActivationFunctionType
AX = mybir.AxisListType


class Cfg:
    def __init__(self, S=256, L=512, E=512, H=4, NC=8, OFF=4, SB=64,
                 no_collective=False, reps=1,
                 skip_band=False, skip_attn=False, nmax=None, tune=None):
        self.S, self.L, self.E, self.H, self.NC = S, L, E, H, NC
        self.CH = L // NC              # L-chunk per core
        self.OFF = OFF                 # partner offset = n1 // CH
        assert OFF * 2 >= NC, "single-partner scheme needs OFF >= NC/2"
        self.n1 = self.n2 = OFF * self.CH
        self.HD = E // H
        assert self.HD == 128 and E % 128 == 0
        self.EB = E // 128             # e partition blocks
        self.SB = SB                   # band s-sub size
        assert S % SB == 0
        self.NSS = S // SB
        self.NST = (S + 127) // 128    # s tiles of <=128 in attention
        self.STW = min(128, S)
        assert self.STW == 128 and self.NST == 2
        self.no_collective = no_collective
        self.reps = reps
        self.skip_band = skip_band
        self.skip_attn = skip_attn
        self.nmax = nmax if nmax is not None else self.CH
        self.tune = dict(ps_a=3, ps_b=4, ps_t=1, dpool=2, scan=3, qkp=6, PT=10,
                         vp=4, osc=3, oo=3)
        if tune: self.tune.update(tune)

    def key(self):
        return (self.S, self.L, self.E, self.H, self.NC, self.OFF, self.SB,
                self.no_collective, self.reps,
                self.skip_band, self.skip_attn, self.nmax,
                tuple(sorted(self.tune.items())))


def build_nc(cfg: Cfg) -> bass.Bass:
    S, L, E, H, NC = cfg.S, cfg.L, cfg.E, cfg.H, cfg.NC
    CH, EB, SB, HD = cfg.CH, cfg.EB, cfg.SB, cfg.HD
    NSS = cfg.NSS
    BW = SB * CH                       # band tile free width
    NST, STW = cfg.NST, cfg.STW
    NPAIR = 2

    nc = bacc.Bacc(None)
    # ---- parameters
    xband = nc.declare_dram_parameter("xband", [E, S, CH], F16, isOutput=False)
    xattn = nc.declare_dram_parameter("xattn", [E, CH, S], F16, isOutput=False)
    xp = nc.declare_dram_parameter("xp", [E, S, CH], F16, isOutput=False)
    wband = nc.declare_dram_parameter("wband", [6, E, CH], F16, isOutput=False)
    gate_in = nc.declare_dram_parameter("gate_in", [128, BW], F16, isOutput=False)
    coef = nc.declare_dram_parameter("coef", [128, 2 * NC], F32, isOutput=False)
    wq = nc.declare_dram_parameter("wq", [E, E], F16, isOutput=False)
    wk = nc.declare_dram_parameter("wk", [E, E], F16, isOutput=False)
    wv = nc.declare_dram_parameter("wv", [E, E], F16, isOutput=False)
    wo = nc.declare_dram_parameter("wo", [E, E], F16, isOutput=False)
    biasv = nc.declare_dram_parameter("biasv", [1, E], F16, isOutput=False)
    wlast = nc.declare_dram_parameter("wlast", [E, 2], F32, isOutput=False)
    biasc = nc.declare_dram_parameter("biasc", [E, 4], F32, isOutput=False)
    ident_in = nc.declare_dram_parameter("ident_in", [128, 128], F16, isOutput=False)
    out = nc.declare_dram_parameter("out", [E, CH, S], F16, isOutput=True)

    # ---- internal DRAM
    qdram = nc.dram_tensor("qdram", [E, CH, S], F16)
    kdram = nc.dram_tensor("kdram", [E, CH, S], F16)
    tin = [nc.dram_tensor(f"tin{eb}", [4, 128, S], F16) for eb in range(EB)]
    tout = [nc.dram_tensor(f"tout{eb}", [4 * NC, 128, S], F16,
                           addr_space="Shared") for eb in range(EB)]

    with tile.TileContext(nc) as tc:
        with (
            tc.tile_pool(name="const", bufs=1) as cpool,
            tc.tile_pool(name="ps_a", bufs=cfg.tune["ps_a"], space="PSUM") as ps_a,
            tc.tile_pool(name="ps_b", bufs=cfg.tune["ps_b"], space="PSUM") as ps_b,
            tc.tile_pool(name="ps_t", bufs=cfg.tune["ps_t"], space="PSUM") as ps_t,
        ):
            # ================= constants =================
            gate = cpool.tile([128, BW], F16, name="gate")
            nc.sync.dma_start(gate[:], gate_in[:, :])
            ident = cpool.tile([128, 128], F16, name="ident")
            nc.sync.dma_start(ident[:], ident_in[:, :])
            coef_sb = cpool.tile([128, 2 * NC], F32, name="coef_sb")
            nc.sync.dma_start(coef_sb[:], coef[:, :])
            biasv_sb = cpool.tile([1, E], F16, name="biasv_sb")
            nc.sync.dma_start(biasv_sb[:], biasv[:, :])
            wlast_sb = []
            for eb in range(EB):
                t = cpool.tile([128, 2], F32, name=f"wlast_{eb}")
                nc.sync.dma_start(t[:], wlast[eb * 128:(eb + 1) * 128, :])
                wlast_sb.append(t)
            biasc_sb = cpool.tile([128, 4 * EB], F32, name="biasc_sb")
            for eb in range(EB):
                nc.sync.dma_start(biasc_sb[:, 4 * eb:4 * (eb + 1)],
                                  biasc[eb * 128:(eb + 1) * 128, :])
            ones_row = cpool.tile([1, max(S, 128)], F16, name="ones_row")
            nc.vector.memset(ones_row[:], 1.0)

            wband_sb = []
            for kind in range(6):
                row = []
                for eb in range(EB):
                    t = cpool.tile([128, CH], F16, name=f"wband_{kind}_{eb}")
                    nc.sync.dma_start(t[:], wband[kind, eb * 128:(eb + 1) * 128, :])
                    row.append(t)
                wband_sb.append(row)

            def load_w(dram, nm):
                tiles = []
                for eb in range(EB):
                    t = cpool.tile([128, E], F16, name=f"{nm}_{eb}")
                    nc.sync.dma_start(t[:], dram[eb * 128:(eb + 1) * 128, :])
                    tiles.append(t)
                return tiles

            wq_sb = load_w(wq, "wq")
            wk_sb = load_w(wk, "wk")
            wv_sb = load_w(wv, "wv")
            wo_sb = load_w(wo, "wo")

            # totals accumulators (per eb, 4 kinds side by side on free axis)
            tin_sb = [cpool.tile([128, 4 * S], F16, name=f"tin_sb{eb}")
                      for eb in range(EB)]

            def emit_band():
                with (
                    tc.tile_pool(name="bin", bufs=2) as binp,
                    tc.tile_pool(name="prod", bufs=4) as ppool,
                    tc.tile_pool(name="scan", bufs=cfg.tune["scan"]) as spool,
                    tc.tile_pool(name="asm", bufs=2) as apool,
                ):
                    for eb in range(EB):
                        er = slice(eb * 128, (eb + 1) * 128)
                        for ss in range(NSS):
                            sr = slice(ss * SB, (ss + 1) * SB)
                            xb = binp.tile([128, BW], F16, name="xb", tag="xb")
                            nc.sync.dma_start(xb[:], xband[er, sr, :])
                            xpb = binp.tile([128, BW], F16, name="xpb", tag="xpb")
                            nc.sync.dma_start(xpb[:], xp[er, sr, :])
                            x3 = xb[:].rearrange("p (s l) -> p s l", l=CH)
                            xp3 = xpb[:].rearrange("p (s l) -> p s l", l=CH)

                            def prod(kind, src3, nm, eng, shifted):
                                p = ppool.tile([128, BW], F16, name=nm,
                                               tag=f"prod_{eng}")
                                p3 = p[:].rearrange("p (s l) -> p s l", l=CH)
                                e = nc.gpsimd if eng == "g" else nc.vector
                                w = wband_sb[kind][eb]
                                if shifted:
                                    e.memset(p3[:, :, 0:1], 0.0)
                                    wb = w[:, 0:CH - 1].unsqueeze(1) \
                                        .broadcast_to([128, SB, CH - 1])
                                    e.tensor_tensor(p3[:, :, 1:CH],
                                                    src3[:, :, 0:CH - 1], wb,
                                                    op=ALU.mult)
                                else:
                                    wb = w[:].unsqueeze(1) \
                                        .broadcast_to([128, SB, CH])
                                    e.tensor_tensor(p3, src3, wb, op=ALU.mult)
                                return p

                            def scan(p, nm):
                                o = spool.tile([128, BW], F16, name=nm, tag="scan")
                                nc.vector.tensor_tensor_scan(
                                    o[:], gate[:], p[:], 0.0,
                                    op0=ALU.mult, op1=ALU.add)
                                return o

                            def half(qk, kf, ks, kp_, dram, store_eng):
                                # kf: fwd kind (shifted->exclusive), ks: bwd
                                # kind (inclusive), kp_: partner kind
                                pf = prod(kf, x3, "pf", "g", True)
                                ps_ = prod(ks, x3, "ps", "g", False)
                                pp = prod(kp_, xp3, "pp", "g", False)
                                # totals via reduces (scan no longer yields
                                # per-kind prefixes): T_f needs the missing
                                # last product added back; T_s is direct
                                koff = kf * S
                                tmp = apool.tile([128, SB], F16, name="tfix",
                                                 tag="tfix")
                                nc.vector.tensor_scalar_mul(
                                    tmp[:], x3[:, :, CH - 1],
                                    wlast_sb[eb][:, kf:kf + 1])
                                tred = apool.tile([128, SB], F32, name="tred",
                                                  tag="tred")
                                nc.vector.tensor_reduce(
                                    tred[:].rearrange("p (s o) -> p s o", o=1),
                                    pf[:].rearrange("p (s l) -> p s l", l=CH),
                                    axis=AX.X, op=ALU.add)
                                nc.vector.tensor_tensor(
                                    tin_sb[eb][:, koff + ss * SB:koff + (ss + 1) * SB],
                                    tred[:], tmp[:], op=ALU.add)
                                with nc.allow_low_precision(
                                        reason="chunk totals fit fp16"):
                                    nc.vector.tensor_reduce(
                                        tin_sb[eb][:, ks * S + ss * SB:ks * S + (ss + 1) * SB]
                                        .rearrange("p (s o) -> p s o", o=1),
                                        ps_[:].rearrange("p (s l) -> p s l", l=CH),
                                        axis=AX.X, op=ALU.add)
                                # combined scan: x + scan(pf + pp - ps)
                                c1 = apool.tile([128, BW], F16, name=f"c1{qk}",
                                                tag="ts")
                                nc.vector.tensor_tensor(c1[:], pf[:], pp[:],
                                                        op=ALU.add)
                                c2 = apool.tile([128, BW], F16, name=f"c2{qk}",
                                                tag="t1")
                                nc.vector.tensor_tensor(c2[:], c1[:], ps_[:],
                                                        op=ALU.subtract)
                                I = scan(c2, "I")
                                t2 = apool.tile([128, BW], F16, name=f"t2{qk}",
                                                tag="ts")
                                nc.vector.tensor_tensor(t2[:], xb[:], I[:],
                                                        op=ALU.add)
                                # free-dim permute (s,l)->(l,s) on Scalar
                                o2 = apool.tile([128, BW], F16, name=f"o2{qk}",
                                                tag="o2")
                                nc.scalar.copy(
                                    o2[:].rearrange("p (l s) -> p l s", s=SB),
                                    t2[:].rearrange("p (s l) -> p l s", l=CH))
                                store_eng.dma_start(
                                    dram[er, 0:CH, sr],
                                    o2[:].rearrange("p (l s) -> p l s", s=SB))

                            half("q", 0, 2, 4, qdram, nc.scalar)
                            half("k", 1, 3, 5, kdram, nc.sync)

                        # per-eb totals store + collective
                        for kind in range(4):
                            nc.sync.dma_start(
                                tin[eb][kind, :, :],
                                tin_sb[eb][:, kind * S:(kind + 1) * S])
                        if not cfg.no_collective:
                            nc.gpsimd.collective_compute(
                                "AllGather", ALU.bypass,
                                replica_groups=[list(range(NC))],
                                ins=[tin[eb][:, :, :]], outs=[tout[eb][:, :, :]],
                            )

            # ================= B-terms =================
            def emit_b():
                with tc.tile_pool(name="bterm", bufs=2) as btp:
                    Bqp, Bkp = [], []  # fp16 [128, S] per fm, proj-space
                    Bq_eb, Bk_eb = [], []
                    for eb in range(EB):
                        for qk, kinds, dst in (("q", (0, 2), Bq_eb),
                                               ("k", (1, 3), Bk_eb)):
                            acc = cpool.tile([128, S], F32, name=f"B{qk}_{eb}")
                            nc.vector.memset(acc[:], 0.0)
                            for j in range(NC):
                                for ci, kind in enumerate(kinds):
                                    tsl = btp.tile([128, S], F16, name="tsl",
                                                   tag="tsl", bufs=3)
                                    nc.sync.dma_start(
                                        tsl[:], tout[eb][4 * j + kind, :, :])
                                    nc.vector.scalar_tensor_tensor(
                                        acc[:], tsl[:],
                                        coef_sb[:, ci * NC + j:ci * NC + j + 1],
                                        acc[:], op0=ALU.mult, op1=ALU.add)
                            h = cpool.tile([128, S], F16, name=f"B16{qk}_{eb}")
                            nc.vector.tensor_copy(h[:], acc[:])
                            dst.append(h)
                    for qk, w_sb, B_eb, bj, dst in (
                            ("q", wq_sb, Bq_eb, 0, Bqp),
                            ("k", wk_sb, Bk_eb, 1, Bkp)):
                        for fm in range(EB):
                            fr = slice(fm * 128, (fm + 1) * 128)
                            acc = ps_a.tile([128, S], F32, name=f"psB{qk}{fm}",
                                            tag="ps_mm")
                            for eb in range(EB):
                                nc.tensor.matmul(acc[:], w_sb[eb][:, fr],
                                                 B_eb[eb][:],
                                                 start=(eb == 0), stop=(eb == EB - 1))
                            o = cpool.tile([128, S], F16, name=f"B{qk}p_{fm}")
                            nc.vector.tensor_scalar_add(
                                o[:], acc[:],
                                biasc_sb[:, 4 * fm + bj:4 * fm + bj + 1])
                            dst.append(o)
                    return Bqp, Bkp

            # ================= attention =================
            def emit_attn(n, qt, kt, xt, Bqp, Bkp, apool):
                T = cfg.tune

                # q/k projections: fm-PAIR tiles [128, 2*S] (one PSUM bank)
                def proj2(w_sb, src, Bp, nm):
                    outt = []
                    for fp_ in range(EB // 2):
                        acc = ps_a.tile([128, 2 * S], F32, name=f"ps{nm}{fp_}",
                                        tag="ps_mm")
                        for half_ in range(2):
                            fm = fp_ * 2 + half_
                            fr = slice(fm * 128, (fm + 1) * 128)
                            hs = slice(half_ * S, (half_ + 1) * S)
                            for eb in range(EB):
                                nc.tensor.matmul(acc[:, hs], w_sb[eb][:, fr],
                                                 src[eb],
                                                 start=(eb == 0), stop=(eb == EB - 1))
                        o = apool.tile([128, 2 * S], F16, name=f"{nm}_{fp_}",
                                       tag="qkp", bufs=T["qkp"])
                        for half_ in range(2):
                            fm = fp_ * 2 + half_
                            hs = slice(half_ * S, (half_ + 1) * S)
                            nc.vector.tensor_tensor(o[:, hs], acc[:, hs],
                                                    Bp[fm][:], op=ALU.add)
                        outt.append(o)
                    return outt

                qp2 = proj2(wq_sb, qt, Bqp, "qp")
                kp2 = proj2(wk_sb, kt, Bkp, "kp")

                def hview(p2, h):
                    # head h lives in pair tile h//2, half h%2
                    return p2[h // 2][:, (h % 2) * S:(h % 2 + 1) * S]

                # v projection: (s, e) tiles
                vp = []
                for st in range(NST):
                    scols = slice(st * 128, st * 128 + STW)
                    acc = ps_a.tile([STW, E], F32, name=f"psv{st}", tag="ps_mm")
                    for eb in range(EB):
                        nc.tensor.matmul(acc[:], xt[eb][:, scols], wv_sb[eb][:],
                                         start=(eb == 0), stop=False)
                    nc.tensor.matmul(acc[:], ones_row[:1, :STW], biasv_sb[:1, :],
                                     start=False, stop=True)
                    o = apool.tile([STW, E], F16, name=f"vp_{st}", tag="vp",
                                   bufs=T["vp"])
                    nc.vector.tensor_copy(o[:], acc[:])
                    vp.append(o)

                # pass 1: (s,t) scores -> -max, den -> -lse (hi/lo fp16)
                nmax_c = apool.tile([STW, 2 * H], F32, name="nmaxc",
                                    tag="nmaxc", bufs=2)
                den_c = apool.tile([STW, 2 * H], F32, name="denc", tag="denc",
                                   bufs=2)
                escr = apool.tile([STW, S], F16, name="escr", tag="escr", bufs=2)
                for st in range(NST):
                    scols = slice(st * 128, st * 128 + STW)
                    for h in range(H):
                        c = h * NST + st
                        accs = ps_b.tile([STW, S], F32, name=f"pssh{st}{h}",
                                         tag="ps_sc")
                        nc.tensor.matmul(accs[:], hview(qp2, h)[:, scols],
                                         hview(kp2, h), start=True, stop=True)
                        nc.vector.tensor_reduce(
                            nmax_c[:, c:c + 1], accs[:], axis=AX.X,
                            op=ALU.max, negate=True)
                        nc.scalar.activation(
                            escr[:], accs[:], ACTF.Exp,
                            bias=nmax_c[:, c:c + 1], scale=1.0,
                            accum_out=den_c[:, c:c + 1])
                ln_c = apool.tile([STW, 2 * H], F32, name="lnc", tag="lnc", bufs=2)
                nc.scalar.activation(ln_c[:], den_c[:], ACTF.Ln)
                lse32 = apool.tile([STW, 2 * H], F32, name="lse32", tag="lse32",
                                   bufs=2)
                nc.vector.tensor_tensor(lse32[:], nmax_c[:], ln_c[:],
                                        op=ALU.subtract)  # -(max+ln den)
                pk = apool.tile([STW, 4 * H], F16, name="lsepack", tag="lsepack",
                                bufs=2)
                nc.vector.tensor_copy(pk[:, 0:2 * H], lse32[:])
                resid = apool.tile([STW, 2 * H], F32, name="resid", tag="resid",
                                   bufs=2)
                nc.vector.tensor_tensor(resid[:], lse32[:], pk[:, 0:2 * H],
                                        op=ALU.subtract)
                nc.vector.tensor_copy(pk[:, 2 * H:4 * H], resid[:])
                # partition->free flatten: lseflat[0, s*16+r] = pk[s, r]
                lseflat = apool.tile([1, STW * 4 * H], F16, name="lseflat",
                                     tag="lseflat", bufs=3)
                nc.sync.dma_start(
                    lseflat[:].rearrange("o (s r) -> o s r", r=4 * H),
                    pk[:])

                # pass 2: scores^T + rank-1(-lse) -> exp -> attn^T (t,s)
                PT = []
                for h in range(H):
                    row = []
                    for tt in range(NST):
                        tcols = slice(tt * 128, tt * 128 + STW)
                        acc = ps_b.tile([STW, S], F32, name=f"psT{h}{tt}",
                                        tag="ps_sc")
                        nc.tensor.matmul(acc[:], hview(kp2, h)[:, tcols],
                                         hview(qp2, h), start=True, stop=False)
                        lse_rs = lseflat[:].rearrange("o (s r) -> o r s",
                                                      r=4 * H)
                        for part in range(2):
                            r0 = part * 2 * H + h * NST
                            nc.tensor.matmul(
                                acc[:], ones_row[:1, :STW],
                                lse_rs[:, r0:r0 + NST, :],
                                start=False, stop=(part == 1))
                        p = apool.tile([STW, S], F16, name=f"PT{h}{tt}",
                                       tag="PT", bufs=T["PT"])
                        nc.scalar.activation(p[:], acc[:], ACTF.Exp)
                        row.append(p)
                    PT.append(row)

                # attn @ V -> o^T (hd, s), h-PAIRS in one PSUM bank
                osc = []
                for hp in range(H // 2):
                    acc = ps_t.tile([HD, 2 * S], F32, name=f"pso{hp}", tag="ps_oo")
                    for half_ in range(2):
                        h = hp * 2 + half_
                        hr = slice(h * HD, (h + 1) * HD)
                        hs = slice(half_ * S, (half_ + 1) * S)
                        for tt in range(NST):
                            nc.tensor.matmul(acc[:, hs], vp[tt][:, hr],
                                             PT[h][tt][:],
                                             start=(tt == 0), stop=(tt == NST - 1))
                    o = apool.tile([HD, 2 * S], F16, name=f"osc{hp}", tag="osc",
                                   bufs=T["osc"])
                    nc.vector.tensor_copy(o[:], acc[:])
                    osc.append(o)

                def oview(f):
                    return osc[f // 2][:, (f % 2) * S:(f % 2 + 1) * S]

                # out projection -> out[g, n, s]
                for gp in range(EB // 2):
                    acc = ps_a.tile([128, 2 * S], F32, name=f"psout{gp}",
                                    tag="ps_mm")
                    for half_ in range(2):
                        gm = gp * 2 + half_
                        gr = slice(gm * 128, (gm + 1) * 128)
                        hs = slice(half_ * S, (half_ + 1) * S)
                        for fm in range(EB):
                            nc.tensor.matmul(acc[:, hs], wo_sb[fm][:, gr],
                                             oview(fm), start=(fm == 0),
                                             stop=(fm == EB - 1))
                    o = apool.tile([128, 2 * S], F16, name=f"oo{gp}", tag="oo",
                                   bufs=T["oo"])
                    for half_ in range(2):
                        gm = gp * 2 + half_
                        hs = slice(half_ * S, (half_ + 1) * S)
                        nc.vector.tensor_scalar_add(
                            o[:, hs], acc[:, hs],
                            biasc_sb[:, 4 * gm + 3:4 * gm + 4])
                    for half_ in range(2):
                        gm = gp * 2 + half_
                        gr = slice(gm * 128, (gm + 1) * 128)
                        hs = slice(half_ * S, (half_ + 1) * S)
                        nc.scalar.dma_start(out[gr, n, :], o[:, hs])

            def emit_attn_all(Bqp, Bkp):
                with (
                    tc.tile_pool(name="dpool", bufs=cfg.tune["dpool"]) as dpool,
                    tc.tile_pool(name="attn", bufs=2) as apool,
                ):
                    NMAX = cfg.nmax if not cfg.skip_attn else 0
                    for n0 in range(0, NMAX, NPAIR):
                        qt2, kt2, xt2 = [], [], []
                        nsl = slice(n0, n0 + NPAIR)
                        for eb in range(EB):
                            er = slice(eb * 128, (eb + 1) * 128)
                            t = dpool.tile([128, NPAIR * S], F16, name=f"qt{eb}",
                                           tag=f"qt{eb}")
                            nc.sync.dma_start(t[:], qdram[er, nsl, :])
                            qt2.append(t)
                            t = dpool.tile([128, NPAIR * S], F16, name=f"kt{eb}",
                                           tag=f"kt{eb}")
                            nc.sync.dma_start(t[:], kdram[er, nsl, :])
                            kt2.append(t)
                            t = dpool.tile([128, NPAIR * S], F16, name=f"xt{eb}",
                                           tag=f"xt{eb}")
                            nc.sync.dma_start(t[:], xattn[er, nsl, :])
                            xt2.append(t)
                        for j in range(NPAIR):
                            if n0 + j >= NMAX:
                                break
                            jsl = slice(j * S, (j + 1) * S)
                            emit_attn(n0 + j,
                                      [t[:, jsl] for t in qt2],
                                      [t[:, jsl] for t in kt2],
                                      [t[:, jsl] for t in xt2],
                                      Bqp, Bkp, apool)

            for _rep in range(cfg.reps):
                if not cfg.skip_band:
                    emit_band()
                Bqp, Bkp = emit_b()
                emit_attn_all(Bqp, Bkp)

    nc.finalize()
    return nc


# ============================================================
# host side
# ============================================================

def prep_inputs(cfg: Cfg, x, a, b, c, d, in_proj_w, in_proj_b, out_w, out_b):
    S, L, E, NC, CH, OFF = cfg.S, cfg.L, cfg.E, cfg.NC, cfg.CH, cfg.OFF
    f32, f16 = np.float32, np.float16
    x = np.asarray(x, f32)
    xg = np.ascontiguousarray(x.transpose(2, 0, 1))     # (E, S, L)
    hd = cfg.HD
    scl = 1.0 / math.sqrt(hd)
    wq = np.ascontiguousarray(in_proj_w[:E].T * scl).astype(f16)
    wk = np.ascontiguousarray(in_proj_w[E:2 * E].T).astype(f16)
    wv = np.ascontiguousarray(in_proj_w[2 * E:].T).astype(f16)
    wo = np.ascontiguousarray(out_w.T).astype(f16)
    bq = in_proj_b[:E] * scl
    bk = in_proj_b[E:2 * E]
    bv = in_proj_b[2 * E:]
    bo = out_b
    biasv = np.asarray(bv, f16).reshape(1, E)
    # last-column fwd weights (per core below)
    biasc = np.ascontiguousarray(
        np.stack([bq, bk, bv, bo]).astype(f32).T)       # (E, 4)
    ident = np.eye(128, dtype=f16)

    gate = np.ones((128, cfg.SB * CH), f16)
    gate[:, ::CH] = 0.0

    in_maps = []
    for k in range(NC):
        chk = slice(CH * k, CH * (k + 1))
        xbandc = np.ascontiguousarray(xg[:, :, chk]).astype(f16)
        xattnc = np.ascontiguousarray(
            xg[:, :, chk].transpose(0, 2, 1)).astype(f16)
        if k >= OFF:
            pf = slice(CH * (k - OFF), CH * (k - OFF + 1))
            xpc = np.ascontiguousarray(xg[:, :, pf]).astype(f16)
            w1 = -a[pf].astype(f32)
            w2 = -b[pf].astype(f32)
        else:
            st = CH * (k + OFF) - 1
            xpc = np.zeros((E, S, CH), f16)
            xpc[:, :, 1:] = xg[:, :, st + 1:st + CH]
            w1 = np.zeros((CH, E), f32)
            w1[1:] = c[st + 1:st + CH]
            w2 = np.zeros((CH, E), f32)
            w2[1:] = d[st + 1:st + CH]
        wbandc = np.ascontiguousarray(
            np.stack([a[chk], b[chk], c[chk], d[chk], w1, w2])
            .transpose(0, 2, 1)).astype(f16)            # (6, E, CH)
        coefA = np.zeros(NC, f32)
        coefA[max(0, k - OFF):k] = 1.0
        coefC = np.zeros(NC, f32)
        coefC[k:min(k + OFF - 1, NC - 1) + 1] = 1.0
        coefv = np.broadcast_to(
            np.concatenate([coefA, coefC])[None, :], (128, 2 * NC)).copy()
        wlastc = np.ascontiguousarray(
            np.stack([a[chk][CH - 1], b[chk][CH - 1]], axis=1)).astype(f32)
        in_maps.append(dict(
            xband=xbandc, xattn=xattnc, xp=xpc,
            wband=wbandc, gate_in=gate, coef=coefv, wlast=wlastc,
            wq=wq, wk=wk, wv=wv, wo=wo, biasv=biasv, biasc=biasc,
            ident_in=ident,
        ))
    return in_maps


_CACHE = {}


def run(cfg: Cfg, inputs, core_ids=None, **kw):
    key = cfg.key()
    if key not in _CACHE:
        _CACHE[key] = build_nc(cfg)
    nc = _CACHE[key]
    in_maps = prep_inputs(
        cfg, inputs["x"], inputs["a"], inputs["b"], inputs["c"], inputs["d"],
        inputs["in_proj_w"], inputs["in_proj_b"], inputs["out_w"], inputs["out_b"])
    res = run_bass_kernel_spmd(nc, in_maps, core_ids or list(range(cfg.NC)), **kw)
    S, L, E, CH = cfg.S, cfg.L, cfg.E, cfg.CH
    full = np.empty((S, L, E), np.float32)
    for k in range(cfg.NC):
        # out is (E, CH, S) fp16
        full[:, CH * k:CH * (k + 1), :] = \
            res.results[k]["out"].astype(np.float32).transpose(2, 1, 0)
    return full, res


def kernel(**inputs) -> np.ndarray:
    assert int(inputs["n1"]) == 256 and int(inputs["n2"]) == 256
    cfg = Cfg()
    out, _ = run(cfg, inputs)
    return out


# revision 28
# speedup vs baseline: 5.5426x; 1.0841x over previous
"""Trainium2 Bass kernel for nn_MultiHeadSSAN: banded Q/K (prefix-sum windows
along feature_len) + multi-head self-attention, sharded over the feature_len
(L) axis across 8 NeuronCores.

v2 design (fp16 end-to-end data path, fp32 PSUM/scan accumulation):

  Band:  per (eb, ss) tile [128e, SB*CH] fp16, s-major l-inner.
         Products x*w split across GpSimd (4) and DVE (2); the fwd products
         are pre-shifted one step along l so a single inclusive
         tensor_tensor_scan yields the exclusive prefix directly (scan state
         is fp32 internally).  Assemble is 3 contiguous fp16 adds; the (s,l)
         -> (l,s) free-dim permute runs on the Scalar engine; stores go out
         as fp16 with 128B runs split across the Scalar and Sync queues.
         Chunk totals are extracted with small strided DVE copies and
         AllGathered per-eb (4 small collectives) so communication overlaps
         the remaining band compute.
  MHA:   per n: all matmuls fp16 (1 cyc/row, FWL weight loads).  Boundary
         constants fold into q/k via the PSUM-evacuation adds (no identity
         matmuls).  Softmax: (s,t) scores give -max (DVE reduce) and den
         (exp accum), lse = -(max+ln den) is split hi/lo fp16, PE-transposed
         to rows, and folded into the transposed (t,s) scores via rank-1
         PSUM accumulates, so the second exp emits normalized attn^T
         directly.  Output is stored fp16 and upcast on host.
"""
import math
import numpy as np

import concourse.bass as bass
import concourse.bacc as bacc
import concourse.mybir as mybir
import concourse.tile as tile
from concourse.bass_utils import run_bass_kernel_spmd

F32 = mybir.dt.float32
F16 = mybir.dt.float16
ALU = mybir.AluOpType
ACTF = mybir.ActivationFunctionType
AX = mybir.AxisListType


class Cfg:
    def __init__(self, S=256, L=512, E=512, H=4, NC=8, OFF=4, SB=64,
                 no_collective=False, reps=1,
                 skip_band=False, skip_attn=False, nmax=None, tune=None):
        self.S, self.L, self.E, self.H, self.NC = S, L, E, H, NC
        self.CH = L // NC              # L-chunk per core
        self.OFF = OFF                 # partner offset = n1 // CH
        assert OFF * 2 >= NC, "single-partner scheme needs OFF >= NC/2"
        self.n1 = self.n2 = OFF * self.CH
        self.HD = E // H
        assert self.HD == 128 and E % 128 == 0
        self.EB = E // 128             # e partition blocks
        self.SB = SB                   # band s-sub size
        assert S % SB == 0
        self.NSS = S // SB
        self.NST = (S + 127) // 128    # s tiles of <=128 in attention
        self.STW = min(128, S)
        assert self.STW == 128 and self.NST == 2
        self.no_collective = no_collective
        self.reps = reps
        self.skip_band = skip_band
        self.skip_attn = skip_attn
        self.nmax = nmax if nmax is not None else self.CH
        self.tune = dict(ps_a=3, ps_b=4, ps_t=1, dpool=2, scan=3, qkp=10, PT=18,
                         vp=6, osc=6, oo=4)
        if tune: self.tune.update(tune)

    def key(self):
        return (self.S, self.L, self.E, self.H, self.NC, self.OFF, self.SB,
                self.no_collective, self.reps,
                self.skip_band, self.skip_attn, self.nmax,
                tuple(sorted(self.tune.items())))


def build_nc(cfg: Cfg) -> bass.Bass:
    S, L, E, H, NC = cfg.S, cfg.L, cfg.E, cfg.H, cfg.NC
    CH, EB, SB, HD = cfg.CH, cfg.EB, cfg.SB, cfg.HD
    NSS = cfg.NSS
    BW = SB * CH                       # band tile free width
    NST, STW = cfg.NST, cfg.STW
    NPAIR = 2

    nc = bacc.Bacc(None)
    # ---- parameters
    xband = nc.declare_dram_parameter("xband", [E, S, CH], F16, isOutput=False)
    xattn = nc.declare_dram_parameter("xattn", [E, CH, S], F16, isOutput=False)
    xp = nc.declare_dram_parameter("xp", [E, S, CH], F16, isOutput=False)
    wband = nc.declare_dram_parameter("wband", [6, E, CH], F16, isOutput=False)
    gate_in = nc.declare_dram_parameter("gate_in", [128, BW], F16, isOutput=False)
    coef = nc.declare_dram_parameter("coef", [128, 2 * NC], F32, isOutput=False)
    wq = nc.declare_dram_parameter("wq", [E, E], F16, isOutput=False)
    wk = nc.declare_dram_parameter("wk", [E, E], F16, isOutput=False)
    wv = nc.declare_dram_parameter("wv", [E, E], F16, isOutput=False)
    wo = nc.declare_dram_parameter("wo", [E, E], F16, isOutput=False)
    biasv = nc.declare_dram_parameter("biasv", [1, E], F16, isOutput=False)
    biasc = nc.declare_dram_parameter("biasc", [E, 4], F32, isOutput=False)
    ident_in = nc.declare_dram_parameter("ident_in", [128, 128], F16, isOutput=False)
    out = nc.declare_dram_parameter("out", [E, CH, S], F16, isOutput=True)

    # ---- internal DRAM
    qdram = nc.dram_tensor("qdram", [E, CH, S], F16)
    kdram = nc.dram_tensor("kdram", [E, CH, S], F16)
    tin = [nc.dram_tensor(f"tin{eb}", [4, 128, S], F16) for eb in range(EB)]
    tout = [nc.dram_tensor(f"tout{eb}", [4 * NC, 128, S], F16,
                           addr_space="Shared") for eb in range(EB)]

    with tile.TileContext(nc) as tc:
        with (
            tc.tile_pool(name="const", bufs=1) as cpool,
            tc.tile_pool(name="ps_a", bufs=cfg.tune["ps_a"], space="PSUM") as ps_a,
            tc.tile_pool(name="ps_b", bufs=cfg.tune["ps_b"], space="PSUM") as ps_b,
            tc.tile_pool(name="ps_t", bufs=cfg.tune["ps_t"], space="PSUM") as ps_t,
        ):
            # ================= constants =================
            gate = cpool.tile([128, BW], F16, name="gate")
            nc.sync.dma_start(gate[:], gate_in[:, :])
            ident = cpool.tile([128, 128], F16, name="ident")
            nc.sync.dma_start(ident[:], ident_in[:, :])
            coef_sb = cpool.tile([128, 2 * NC], F32, name="coef_sb")
            nc.sync.dma_start(coef_sb[:], coef[:, :])
            biasv_sb = cpool.tile([1, E], F16, name="biasv_sb")
            nc.sync.dma_start(biasv_sb[:], biasv[:, :])
            biasc_sb = cpool.tile([128, 4 * EB], F32, name="biasc_sb")
            for eb in range(EB):
                nc.sync.dma_start(biasc_sb[:, 4 * eb:4 * (eb + 1)],
                                  biasc[eb * 128:(eb + 1) * 128, :])
            ones_row = cpool.tile([1, max(S, 128)], F16, name="ones_row")
            nc.vector.memset(ones_row[:], 1.0)

            wband_sb = []
            for kind in range(6):
                row = []
                for eb in range(EB):
                    t = cpool.tile([128, CH], F16, name=f"wband_{kind}_{eb}")
                    nc.sync.dma_start(t[:], wband[kind, eb * 128:(eb + 1) * 128, :])
                    row.append(t)
                wband_sb.append(row)

            def load_w(dram, nm):
                tiles = []
                for eb in range(EB):
                    t = cpool.tile([128, E], F16, name=f"{nm}_{eb}")
                    nc.sync.dma_start(t[:], dram[eb * 128:(eb + 1) * 128, :])
                    tiles.append(t)
                return tiles

            wq_sb = load_w(wq, "wq")
            wk_sb = load_w(wk, "wk")
            wv_sb = load_w(wv, "wv")
            wo_sb = load_w(wo, "wo")

            # totals accumulators (per eb, 4 kinds side by side on free axis)
            tin_sb = [cpool.tile([128, 4 * S], F16, name=f"tin_sb{eb}")
                      for eb in range(EB)]

            def emit_band():
                with (
                    tc.tile_pool(name="bin", bufs=2) as binp,
                    tc.tile_pool(name="prod", bufs=4) as ppool,
                    tc.tile_pool(name="scan", bufs=cfg.tune["scan"]) as spool,
                    tc.tile_pool(name="asm", bufs=2) as apool,
                ):
                    for eb in range(EB):
                        er = slice(eb * 128, (eb + 1) * 128)
                        for ss in range(NSS):
                            sr = slice(ss * SB, (ss + 1) * SB)
                            xb = binp.tile([128, BW], F16, name="xb", tag="xb")
                            nc.sync.dma_start(xb[:], xband[er, sr, :])
                            xpb = binp.tile([128, BW], F16, name="xpb", tag="xpb")
                            nc.sync.dma_start(xpb[:], xp[er, sr, :])
                            x3 = xb[:].rearrange("p (s l) -> p s l", l=CH)
                            xp3 = xpb[:].rearrange("p (s l) -> p s l", l=CH)

                            def prod(kind, src3, nm, eng):
                                p = ppool.tile([128, BW], F16, name=nm,
                                               tag=f"prod_{eng}")
                                p3 = p[:].rearrange("p (s l) -> p s l", l=CH)
                                e = nc.gpsimd if eng == "g" else nc.vector
                                wb = wband_sb[kind][eb][:].unsqueeze(1) \
                                    .broadcast_to([128, SB, CH])
                                e.tensor_tensor(p3, src3, wb, op=ALU.mult)
                                return p

                            def scan(p, nm):
                                o = spool.tile([128, BW], F16, name=nm, tag="scan")
                                nc.vector.tensor_tensor_scan(
                                    o[:], gate[:], p[:], 0.0,
                                    op0=ALU.mult, op1=ALU.add)
                                return o

                            def half(qk, kf, ks, kp_, dram, store_eng):
                                # kf: fwd kind, ks: bwd kind, kp_: partner
                                pf = prod(kf, x3, "pf", "g")
                                ps_ = prod(ks, x3, "ps", "g")
                                pp = prod(kp_, xp3, "pp", "g")
                                # chunk totals = full-product reduces
                                with nc.allow_low_precision(
                                        reason="chunk totals fit fp16"):
                                    nc.vector.tensor_reduce(
                                        tin_sb[eb][:, kf * S + ss * SB:kf * S + (ss + 1) * SB]
                                        .rearrange("p (s o) -> p s o", o=1),
                                        pf[:].rearrange("p (s l) -> p s l", l=CH),
                                        axis=AX.X, op=ALU.add)
                                    nc.vector.tensor_reduce(
                                        tin_sb[eb][:, ks * S + ss * SB:ks * S + (ss + 1) * SB]
                                        .rearrange("p (s o) -> p s o", o=1),
                                        ps_[:].rearrange("p (s l) -> p s l", l=CH),
                                        axis=AX.X, op=ALU.add)
                                # combined scan of (shift(pf) + pp - ps); the
                                # fwd shift is applied while combining
                                c1 = apool.tile([128, BW], F16, name=f"c1{qk}",
                                                tag="ts")
                                c13 = c1[:].rearrange("p (s l) -> p s l", l=CH)
                                pf3 = pf[:].rearrange("p (s l) -> p s l", l=CH)
                                pp3 = pp[:].rearrange("p (s l) -> p s l", l=CH)
                                nc.vector.tensor_tensor(
                                    c13[:, :, 1:CH], pf3[:, :, 0:CH - 1],
                                    pp3[:, :, 1:CH], op=ALU.add)
                                nc.vector.tensor_copy(c13[:, :, 0:1],
                                                      pp3[:, :, 0:1])
                                c2 = apool.tile([128, BW], F16, name=f"c2{qk}",
                                                tag="t1")
                                nc.vector.tensor_tensor(c2[:], c1[:], ps_[:],
                                                        op=ALU.subtract)
                                I = scan(c2, "I")
                                t2 = apool.tile([128, BW], F16, name=f"t2{qk}",
                                                tag="ts")
                                nc.vector.tensor_tensor(t2[:], xb[:], I[:],
                                                        op=ALU.add)
                                # free-dim permute (s,l)->(l,s) on Scalar
                                o2 = apool.tile([128, BW], F16, name=f"o2{qk}",
                                                tag="o2")
                                nc.scalar.copy(
                                    o2[:].rearrange("p (l s) -> p l s", s=SB),
                                    t2[:].rearrange("p (s l) -> p l s", l=CH))
                                store_eng.dma_start(
                                    dram[er, 0:CH, sr],
                                    o2[:].rearrange("p (l s) -> p l s", s=SB))

                            half("q", 0, 2, 4, qdram, nc.scalar)
                            half("k", 1, 3, 5, kdram, nc.sync)

                        # per-eb totals store + collective
                        for kind in range(4):
                            nc.sync.dma_start(
                                tin[eb][kind, :, :],
                                tin_sb[eb][:, kind * S:(kind + 1) * S])
                        if not cfg.no_collective:
                            nc.gpsimd.collective_compute(
                                "AllGather", ALU.bypass,
                                replica_groups=[list(range(NC))],
                                ins=[tin[eb][:, :, :]], outs=[tout[eb][:, :, :]],
                            )

            # ================= B-terms =================
            def emit_b():
                with tc.tile_pool(name="bterm", bufs=2) as btp:
                    Bqp, Bkp = [], []  # fp16 [128, S] per fm, proj-space
                    Bq_eb, Bk_eb = [], []
                    for eb in range(EB):
                        for qk, kinds, dst in (("q", (0, 2), Bq_eb),
                                               ("k", (1, 3), Bk_eb)):
                            acc = cpool.tile([128, S], F32, name=f"B{qk}_{eb}")
                            nc.vector.memset(acc[:], 0.0)
                            for j in range(NC):
                                for ci, kind in enumerate(kinds):
                                    tsl = btp.tile([128, S], F16, name="tsl",
                                                   tag="tsl", bufs=3)
                                    nc.sync.dma_start(
                                        tsl[:], tout[eb][4 * j + kind, :, :])
                                    nc.vector.scalar_tensor_tensor(
                                        acc[:], tsl[:],
                                        coef_sb[:, ci * NC + j:ci * NC + j + 1],
                                        acc[:], op0=ALU.mult, op1=ALU.add)
                            h = cpool.tile([128, S], F16, name=f"B16{qk}_{eb}")
                            nc.vector.tensor_copy(h[:], acc[:])
                            dst.append(h)
                    for qk, w_sb, B_eb, bj, dst in (
                            ("q", wq_sb, Bq_eb, 0, Bqp),
                            ("k", wk_sb, Bk_eb, 1, Bkp)):
                        for fm in range(EB):
                            fr = slice(fm * 128, (fm + 1) * 128)
                            acc = ps_a.tile([128, S], F32, name=f"psB{qk}{fm}",
                                            tag="ps_mm")
                            for eb in range(EB):
                                nc.tensor.matmul(acc[:], w_sb[eb][:, fr],
                                                 B_eb[eb][:],
                                                 start=(eb == 0), stop=(eb == EB - 1))
                            o = cpool.tile([128, S], F16, name=f"B{qk}p_{fm}")
                            nc.vector.tensor_scalar_add(
                                o[:], acc[:],
                                biasc_sb[:, 4 * fm + bj:4 * fm + bj + 1])
                            dst.append(o)
                    return Bqp, Bkp

            # ================= attention =================
            def emit_attn_pair(n0, qt2, kt2, xt2, Bqp, Bkp, apool):
                T = cfg.tune
                NP = NPAIR

                # q/k projections: per fm(==head), both n at once (N=NP*S)
                def proj(w_sb, src2, Bp, nm):
                    outt = []
                    for fm in range(EB):
                        fr = slice(fm * 128, (fm + 1) * 128)
                        acc = ps_a.tile([128, NP * S], F32, name=f"ps{nm}{fm}",
                                        tag="ps_mm")
                        for eb in range(EB):
                            nc.tensor.matmul(acc[:], w_sb[eb][:, fr],
                                             src2[eb][:],
                                             start=(eb == 0), stop=(eb == EB - 1))
                        o = apool.tile([128, NP * S], F16, name=f"{nm}_{fm}",
                                       tag="qkp", bufs=T["qkp"])
                        for j in range(NP):
                            js = slice(j * S, (j + 1) * S)
                            nc.vector.tensor_tensor(o[:, js], acc[:, js],
                                                    Bp[fm][:], op=ALU.add)
                        outt.append(o)
                    return outt

                qp = proj(wq_sb, qt2, Bqp, "qp")
                kp = proj(wk_sb, kt2, Bkp, "kp")

                def hv(p, h, j):
                    return p[h][:, j * S:(j + 1) * S]

                # v projections per (j, st): (s, e) tiles
                vp = [[None] * NST for _ in range(NP)]
                for j in range(NP):
                    for st in range(NST):
                        scols = slice(j * S + st * 128, j * S + st * 128 + STW)
                        acc = ps_a.tile([STW, E], F32, name=f"psv{j}{st}",
                                        tag="ps_mm")
                        for eb in range(EB):
                            nc.tensor.matmul(acc[:], xt2[eb][:, scols],
                                             wv_sb[eb][:],
                                             start=(eb == 0), stop=False)
                        nc.tensor.matmul(acc[:], ones_row[:1, :STW],
                                         biasv_sb[:1, :], start=False, stop=True)
                        o = apool.tile([STW, E], F16, name=f"vp{j}{st}",
                                       tag="vp", bufs=T["vp"])
                        nc.vector.tensor_copy(o[:], acc[:])
                        vp[j][st] = o

                # pass 1 per j: (s,t) scores -> -max, den
                nmax_c, den_c = [], []
                escr = apool.tile([STW, S], F16, name="escr", tag="escr", bufs=2)
                for j in range(NP):
                    nm_ = apool.tile([STW, 2 * H], F32, name=f"nmaxc{j}",
                                     tag="nmaxc", bufs=4)
                    dn_ = apool.tile([STW, 2 * H], F32, name=f"denc{j}",
                                     tag="denc", bufs=4)
                    nmax_c.append(nm_); den_c.append(dn_)
                    for st in range(NST):
                        scols = slice(st * 128, st * 128 + STW)
                        for h in range(H):
                            c = h * NST + st
                            accs = ps_b.tile([STW, S], F32, name=f"ps1{j}{st}{h}",
                                             tag="ps_sc")
                            nc.tensor.matmul(accs[:], hv(qp, h, j)[:, scols],
                                             hv(kp, h, j), start=True, stop=True)
                            nc.vector.tensor_reduce(
                                nm_[:, c:c + 1], accs[:], axis=AX.X,
                                op=ALU.max, negate=True)
                            nc.scalar.activation(
                                escr[:], accs[:], ACTF.Exp,
                                bias=nm_[:, c:c + 1], scale=1.0,
                                accum_out=dn_[:, c:c + 1])
                # lse chains (Ln ops adjacent on the scalar queue)
                ln_c = []
                for j in range(NP):
                    l_ = apool.tile([STW, 2 * H], F32, name=f"lnc{j}",
                                    tag="lnc", bufs=4)
                    nc.scalar.activation(l_[:], den_c[j][:], ACTF.Ln)
                    ln_c.append(l_)
                lseflat = []
                for j in range(NP):
                    lse32 = apool.tile([STW, 2 * H], F32, name=f"lse32{j}",
                                       tag="lse32", bufs=4)
                    nc.vector.tensor_tensor(lse32[:], nmax_c[j][:], ln_c[j][:],
                                            op=ALU.subtract)  # -(max+ln den)
                    pk = apool.tile([STW, 4 * H], F16, name=f"lsepack{j}",
                                    tag="lsepack", bufs=4)
                    nc.vector.tensor_copy(pk[:, 0:2 * H], lse32[:])
                    resid = apool.tile([STW, 2 * H], F32, name=f"resid{j}",
                                       tag="resid", bufs=4)
                    nc.vector.tensor_tensor(resid[:], lse32[:], pk[:, 0:2 * H],
                                            op=ALU.subtract)
                    nc.vector.tensor_copy(pk[:, 2 * H:4 * H], resid[:])
                    # partition->free flatten: lf[0, s*16+r] = pk[s, r]
                    lf = apool.tile([1, STW * 4 * H], F16, name=f"lseflat{j}",
                                    tag="lseflat", bufs=4)
                    nc.sync.dma_start(
                        lf[:].rearrange("o (s r) -> o s r", r=4 * H), pk[:])
                    lseflat.append(lf)

                # pass 2 per j: scores^T + rank-1(-lse) -> exp -> attn^T
                PT = [[[None] * NST for _ in range(H)] for _ in range(NP)]
                for j in range(NP):
                    lse_rs = lseflat[j][:].rearrange("o (s r) -> o r s",
                                                     r=4 * H)
                    for grp in range(2):
                        accs2 = []
                        for h2 in range(2):
                            h = grp * 2 + h2
                            for tt in range(NST):
                                tcols = slice(tt * 128, tt * 128 + STW)
                                acc = ps_b.tile([STW, S], F32,
                                                name=f"ps2{j}{h}{tt}",
                                                tag="ps_sc")
                                nc.tensor.matmul(acc[:], hv(kp, h, j)[:, tcols],
                                                 hv(qp, h, j),
                                                 start=True, stop=False)
                                accs2.append((acc, h, tt))
                        for acc, h, tt in accs2:
                            for part in range(2):
                                r0 = part * 2 * H + h * NST
                                nc.tensor.matmul(
                                    acc[:], ones_row[:1, :STW],
                                    lse_rs[:, r0:r0 + NST, :],
                                    start=False, stop=(part == 1))
                            p = apool.tile([STW, S], F16, name=f"PT{j}{h}{tt}",
                                           tag="PT", bufs=T["PT"])
                            nc.scalar.activation(p[:], acc[:], ACTF.Exp)
                            PT[j][h][tt] = p

                # attn @ V -> o^T (hd, (n, s)) per head
                osc = []
                for h in range(H):
                    hr = slice(h * HD, (h + 1) * HD)
                    acc = ps_t.tile([HD, NP * S], F32, name=f"pso{h}",
                                    tag="ps_oo")
                    for j in range(NP):
                        js = slice(j * S, (j + 1) * S)
                        for tt in range(NST):
                            nc.tensor.matmul(acc[:, js], vp[j][tt][:, hr],
                                             PT[j][h][tt][:],
                                             start=(tt == 0), stop=(tt == NST - 1))
                    o = apool.tile([HD, NP * S], F16, name=f"osc{h}", tag="osc",
                                   bufs=T["osc"])
                    nc.vector.tensor_copy(o[:], acc[:])
                    osc.append(o)

                # out projection -> out[g, n0:n0+2, s]
                for gm in range(EB):
                    gr = slice(gm * 128, (gm + 1) * 128)
                    acc = ps_a.tile([128, NP * S], F32, name=f"psout{gm}",
                                    tag="ps_mm")
                    for fm in range(EB):
                        nc.tensor.matmul(acc[:], wo_sb[fm][:, gr], osc[fm][:],
                                         start=(fm == 0), stop=(fm == EB - 1))
                    o = apool.tile([128, NP * S], F16, name=f"oo{gm}", tag="oo",
                                   bufs=T["oo"])
                    nc.vector.tensor_scalar_add(
                        o[:], acc[:], biasc_sb[:, 4 * gm + 3:4 * gm + 4])
                    nc.scalar.dma_start(
                        out[gr, n0:n0 + NP, :],
                        o[:].rearrange("p (j s) -> p j s", j=NP))

            def emit_attn_all(Bqp, Bkp):
                with (
                    tc.tile_pool(name="dpool", bufs=cfg.tune["dpool"]) as dpool,
                    tc.tile_pool(name="attn", bufs=2) as apool,
                ):
                    NMAX = cfg.nmax if not cfg.skip_attn else 0
                    assert NMAX % NPAIR == 0
                    for n0 in range(0, NMAX, NPAIR):
                        qt2, kt2, xt2 = [], [], []
                        nsl = slice(n0, n0 + NPAIR)
                        for eb in range(EB):
                            er = slice(eb * 128, (eb + 1) * 128)
                            t = dpool.tile([128, NPAIR * S], F16, name=f"qt{eb}",
                                           tag=f"qt{eb}")
                            nc.sync.dma_start(t[:], qdram[er, nsl, :])
                            qt2.append(t)
                            t = dpool.tile([128, NPAIR * S], F16, name=f"kt{eb}",
                                           tag=f"kt{eb}")
                            nc.sync.dma_start(t[:], kdram[er, nsl, :])
                            kt2.append(t)
                            t = dpool.tile([128, NPAIR * S], F16, name=f"xt{eb}",
                                           tag=f"xt{eb}")
                            nc.sync.dma_start(t[:], xattn[er, nsl, :])
                            xt2.append(t)
                        emit_attn_pair(n0, qt2, kt2, xt2, Bqp, Bkp, apool)

            for _rep in range(cfg.reps):
                if not cfg.skip_band:
                    emit_band()
                Bqp, Bkp = emit_b()
                emit_attn_all(Bqp, Bkp)

    nc.finalize()
    return nc


# ============================================================
# host side
# ============================================================

def prep_inputs(cfg: Cfg, x, a, b, c, d, in_proj_w, in_proj_b, out_w, out_b):
    S, L, E, NC, CH, OFF = cfg.S, cfg.L, cfg.E, cfg.NC, cfg.CH, cfg.OFF
    f32, f16 = np.float32, np.float16
    x = np.asarray(x, f32)
    xg = np.ascontiguousarray(x.transpose(2, 0, 1))     # (E, S, L)
    hd = cfg.HD
    scl = 1.0 / math.sqrt(hd)
    wq = np.ascontiguousarray(in_proj_w[:E].T * scl).astype(f16)
    wk = np.ascontiguousarray(in_proj_w[E:2 * E].T).astype(f16)
    wv = np.ascontiguousarray(in_proj_w[2 * E:].T).astype(f16)
    wo = np.ascontiguousarray(out_w.T).astype(f16)
    bq = in_proj_b[:E] * scl
    bk = in_proj_b[E:2 * E]
    bv = in_proj_b[2 * E:]
    bo = out_b
    biasv = np.asarray(bv, f16).reshape(1, E)
    # last-column fwd weights (per core below)
    biasc = np.ascontiguousarray(
        np.stack([bq, bk, bv, bo]).astype(f32).T)       # (E, 4)
    ident = np.eye(128, dtype=f16)

    gate = np.ones((128, cfg.SB * CH), f16)
    gate[:, ::CH] = 0.0

    in_maps = []
    for k in range(NC):
        chk = slice(CH * k, CH * (k + 1))
        xbandc = np.ascontiguousarray(xg[:, :, chk]).astype(f16)
        xattnc = np.ascontiguousarray(
            xg[:, :, chk].transpose(0, 2, 1)).astype(f16)
        if k >= OFF:
            pf = slice(CH * (k - OFF), CH * (k - OFF + 1))
            xpc = np.ascontiguousarray(xg[:, :, pf]).astype(f16)
            w1 = -a[pf].astype(f32)
            w2 = -b[pf].astype(f32)
        else:
            st = CH * (k + OFF) - 1
            xpc = np.zeros((E, S, CH), f16)
            xpc[:, :, 1:] = xg[:, :, st + 1:st + CH]
            w1 = np.zeros((CH, E), f32)
            w1[1:] = c[st + 1:st + CH]
            w2 = np.zeros((CH, E), f32)
            w2[1:] = d[st + 1:st + CH]
        wbandc = np.ascontiguousarray(
            np.stack([a[chk], b[chk], c[chk], d[chk], w1, w2])
            .transpose(0, 2, 1)).astype(f16)            # (6, E, CH)
        coefA = np.zeros(NC, f32)
        coefA[max(0, k - OFF):k] = 1.0
        coefC = np.zeros(NC, f32)
        coefC[k:min(k + OFF - 1, NC - 1) + 1] = 1.0
        coefv = np.broadcast_to(
            np.concatenate([coefA, coefC])[None, :], (128, 2 * NC)).copy()
        in_maps.append(dict(
            xband=xbandc, xattn=xattnc, xp=xpc,
            wband=wbandc, gate_in=gate, coef=coefv,
            wq=wq, wk=wk, wv=wv, wo=wo, biasv=biasv, biasc=biasc,
            ident_in=ident,
        ))
    return in_maps


_CACHE = {}


def run(cfg: Cfg, inputs, core_ids=None, **kw):
    key = cfg.key()
    if key not in _CACHE:
        _CACHE[key] = build_nc(cfg)
    nc = _CACHE[key]
    in_maps = prep_inputs(
        cfg, inputs["x"], inputs["a"], inputs["b"], inputs["c"], inputs["d"],
        inputs["in_proj_w"], inputs["in_proj_b"], inputs["out_w"], inputs["out_b"])
    res = run_bass_kernel_spmd(nc, in_maps, core_ids or list(range(cfg.NC)), **kw)
    S, L, E, CH = cfg.S, cfg.L, cfg.E, cfg.CH
    full = np.empty((S, L, E), np.float32)
    for k in range(cfg.NC):
        # out is (E, CH, S) fp16
        full[:, CH * k:CH * (k + 1), :] = \
            res.results[k]["out"].astype(np.float32).transpose(2, 1, 0)
    return full, res


def kernel(**inputs) -> np.ndarray:
    assert int(inputs["n1"]) == 256 and int(inputs["n2"]) == 256
    cfg = Cfg()
    out, _ = run(cfg, inputs)
    return out


# revision 36
# speedup vs baseline: 6.3099x; 1.1384x over previous
"""Trainium2 Bass kernel for nn_MultiHeadSSAN: banded Q/K (prefix-sum windows
along feature_len) + multi-head self-attention, sharded over the feature_len
(L) axis across 8 NeuronCores.

v2 design (fp16 end-to-end data path, fp32 PSUM/scan accumulation):

  Band:  per (eb, ss) tile [128e, SB*CH] fp16, s-major l-inner.
         Products x*w split across GpSimd (4) and DVE (2); the fwd products
         are pre-shifted one step along l so a single inclusive
         tensor_tensor_scan yields the exclusive prefix directly (scan state
         is fp32 internally).  Assemble is 3 contiguous fp16 adds; the (s,l)
         -> (l,s) free-dim permute runs on the Scalar engine; stores go out
         as fp16 with 128B runs split across the Scalar and Sync queues.
         Chunk totals are extracted with small strided DVE copies and
         AllGathered per-eb (4 small collectives) so communication overlaps
         the remaining band compute.
  MHA:   per n: all matmuls fp16 (1 cyc/row, FWL weight loads).  Boundary
         constants fold into q/k via the PSUM-evacuation adds (no identity
         matmuls).  Softmax: (s,t) scores give -max (DVE reduce) and den
         (exp accum), lse = -(max+ln den) is split hi/lo fp16, PE-transposed
         to rows, and folded into the transposed (t,s) scores via rank-1
         PSUM accumulates, so the second exp emits normalized attn^T
         directly.  Output is stored fp16 and upcast on host.
"""
import math
import numpy as np

import concourse.bass as bass
import concourse.bacc as bacc
import concourse.mybir as mybir
import concourse.tile as tile
from concourse.bass_utils import run_bass_kernel_spmd

F32 = mybir.dt.float32
F16 = mybir.dt.float16
ALU = mybir.AluOpType
ACTF = mybir.ActivationFunctionType
AX = mybir.AxisListType


class Cfg:
    def __init__(self, S=256, L=512, E=512, H=4, NC=8, OFF=4, SB=64,
                 no_collective=False, reps=1,
                 skip_band=False, skip_attn=False, nmax=None, tune=None):
        self.S, self.L, self.E, self.H, self.NC = S, L, E, H, NC
        self.CH = L // NC              # L-chunk per core
        self.OFF = OFF                 # partner offset = n1 // CH
        assert OFF * 2 >= NC, "single-partner scheme needs OFF >= NC/2"
        self.n1 = self.n2 = OFF * self.CH
        self.HD = E // H
        assert self.HD == 128 and E % 128 == 0
        self.EB = E // 128             # e partition blocks
        self.SB = SB                   # band s-sub size
        assert S % SB == 0
        self.NSS = S // SB
        self.NST = (S + 127) // 128    # s tiles of <=128 in attention
        self.STW = min(128, S)
        assert self.STW == 128 and self.NST == 2
        self.no_collective = no_collective
        self.reps = reps
        self.skip_band = skip_band
        self.skip_attn = skip_attn
        self.nmax = nmax if nmax is not None else self.CH
        self.tune = dict(ps_a=3, ps_b=4, ps_t=1, dpool=3, scan=3, qkp=18, PT=18,
                         vp=10, osc=6, oo=4, lseflat=6)
        if tune: self.tune.update(tune)

    def key(self):
        return (self.S, self.L, self.E, self.H, self.NC, self.OFF, self.SB,
                self.no_collective, self.reps,
                self.skip_band, self.skip_attn, self.nmax,
                tuple(sorted(self.tune.items())))


def build_nc(cfg: Cfg) -> bass.Bass:
    S, L, E, H, NC = cfg.S, cfg.L, cfg.E, cfg.H, cfg.NC
    CH, EB, SB, HD = cfg.CH, cfg.EB, cfg.SB, cfg.HD
    NSS = cfg.NSS
    BW = SB * CH                       # band tile free width
    NST, STW = cfg.NST, cfg.STW
    NPAIR = 2

    nc = bacc.Bacc(None)
    # ---- parameters
    xband = nc.declare_dram_parameter("xband", [E, S, CH], F16, isOutput=False)
    xattn = nc.declare_dram_parameter("xattn", [E, CH, S], F16, isOutput=False)
    xp = nc.declare_dram_parameter("xp", [E, S, CH], F16, isOutput=False)
    wband = nc.declare_dram_parameter("wband", [6, E, CH], F16, isOutput=False)
    gate_in = nc.declare_dram_parameter("gate_in", [128, BW], F16, isOutput=False)
    bqin = nc.declare_dram_parameter("bqin", [E, S], F16, isOutput=False)
    bkin = nc.declare_dram_parameter("bkin", [E, S], F16, isOutput=False)
    wq = nc.declare_dram_parameter("wq", [E, E], F16, isOutput=False)
    wk = nc.declare_dram_parameter("wk", [E, E], F16, isOutput=False)
    wv = nc.declare_dram_parameter("wv", [E, E], F16, isOutput=False)
    wo = nc.declare_dram_parameter("wo", [E, E], F16, isOutput=False)
    biasv = nc.declare_dram_parameter("biasv", [1, E], F16, isOutput=False)
    biasc = nc.declare_dram_parameter("biasc", [E, 4], F32, isOutput=False)
    ident_in = nc.declare_dram_parameter("ident_in", [128, 128], F16, isOutput=False)
    out = nc.declare_dram_parameter("out", [E, CH, S], F16, isOutput=True)

    # ---- internal DRAM
    qdram = nc.dram_tensor("qdram", [E, CH, S], F16)
    kdram = nc.dram_tensor("kdram", [E, CH, S], F16)

    with tile.TileContext(nc) as tc:
        with (
            tc.tile_pool(name="const", bufs=1) as cpool,
            tc.tile_pool(name="ps_a", bufs=cfg.tune["ps_a"], space="PSUM") as ps_a,
            tc.tile_pool(name="ps_b", bufs=cfg.tune["ps_b"], space="PSUM") as ps_b,
            tc.tile_pool(name="ps_t", bufs=cfg.tune["ps_t"], space="PSUM") as ps_t,
        ):
            # ================= constants =================
            gate = cpool.tile([128, BW], F16, name="gate")
            nc.sync.dma_start(gate[:], gate_in[:, :])
            ident = cpool.tile([128, 128], F16, name="ident")
            nc.sync.dma_start(ident[:], ident_in[:, :])

            biasv_sb = cpool.tile([1, E], F16, name="biasv_sb")
            nc.sync.dma_start(biasv_sb[:], biasv[:, :])
            biasc_sb = cpool.tile([128, 4 * EB], F32, name="biasc_sb")
            for eb in range(EB):
                nc.sync.dma_start(biasc_sb[:, 4 * eb:4 * (eb + 1)],
                                  biasc[eb * 128:(eb + 1) * 128, :])
            ones_row = cpool.tile([1, max(S, 128)], F16, name="ones_row")
            nc.vector.memset(ones_row[:], 1.0)

            wband_sb = []
            for kind in range(6):
                row = []
                for eb in range(EB):
                    t = cpool.tile([128, CH], F16, name=f"wband_{kind}_{eb}")
                    nc.sync.dma_start(t[:], wband[kind, eb * 128:(eb + 1) * 128, :])
                    row.append(t)
                wband_sb.append(row)

            def load_w(dram, nm):
                tiles = []
                for eb in range(EB):
                    t = cpool.tile([128, E], F16, name=f"{nm}_{eb}")
                    nc.sync.dma_start(t[:], dram[eb * 128:(eb + 1) * 128, :])
                    tiles.append(t)
                return tiles

            wq_sb = load_w(wq, "wq")
            wk_sb = load_w(wk, "wk")
            wv_sb = load_w(wv, "wv")
            wo_sb = load_w(wo, "wo")

            # host-computed boundary terms per eb
            bq_eb, bk_eb = [], []
            for eb in range(EB):
                er = slice(eb * 128, (eb + 1) * 128)
                t = cpool.tile([128, S], F16, name=f"bq_{eb}")
                nc.sync.dma_start(t[:], bqin[er, :])
                bq_eb.append(t)
                t = cpool.tile([128, S], F16, name=f"bk_{eb}")
                nc.sync.dma_start(t[:], bkin[er, :])
                bk_eb.append(t)

            def emit_band():
                with (
                    tc.tile_pool(name="bin", bufs=2) as binp,
                    tc.tile_pool(name="prod", bufs=5) as ppool,
                    tc.tile_pool(name="scan", bufs=cfg.tune["scan"]) as spool,
                    tc.tile_pool(name="asm", bufs=2) as apool,
                ):
                    for eb in range(EB):
                        er = slice(eb * 128, (eb + 1) * 128)
                        for ss in range(NSS):
                            sr = slice(ss * SB, (ss + 1) * SB)
                            xb = binp.tile([128, BW], F16, name="xb", tag="xb")
                            nc.sync.dma_start(xb[:], xband[er, sr, :])
                            xpb = binp.tile([128, BW], F16, name="xpb", tag="xpb")
                            nc.sync.dma_start(xpb[:], xp[er, sr, :])
                            x3 = xb[:].rearrange("p (s l) -> p s l", l=CH)
                            xp3 = xpb[:].rearrange("p (s l) -> p s l", l=CH)

                            def prod(kind, src3, nm, eng):
                                p = ppool.tile([128, BW], F16, name=nm,
                                               tag=f"prod_{eng}",
                                               bufs=(4 if eng == "g" else 2))
                                p3 = p[:].rearrange("p (s l) -> p s l", l=CH)
                                e = nc.gpsimd if eng == "g" else nc.vector
                                wb = wband_sb[kind][eb][:].unsqueeze(1) \
                                    .broadcast_to([128, SB, CH])
                                e.tensor_tensor(p3, src3, wb, op=ALU.mult)
                                return p

                            def scan(p, nm):
                                o = spool.tile([128, BW], F16, name=nm, tag="scan")
                                nc.vector.tensor_tensor_scan(
                                    o[:], gate[:], p[:], 0.0,
                                    op0=ALU.mult, op1=ALU.add)
                                return o

                            def half(qk, kf, ks, kp_, dram, store_eng):
                                # kf: fwd kind, ks: bwd kind, kp_: partner
                                peng = "v" if qk == "q" else "g"
                                pf = prod(kf, x3, "pf", "g")
                                ps_ = prod(ks, x3, "ps", "g")
                                pp = prod(kp_, xp3, "pp", peng)
                                # combined scan of (shift(pf) + pp - ps); the
                                # fwd shift is applied while combining
                                c1 = apool.tile([128, BW], F16, name=f"c1{qk}",
                                                tag="ts")
                                c13 = c1[:].rearrange("p (s l) -> p s l", l=CH)
                                pf3 = pf[:].rearrange("p (s l) -> p s l", l=CH)
                                pp3 = pp[:].rearrange("p (s l) -> p s l", l=CH)
                                nc.vector.tensor_tensor(
                                    c13[:, :, 1:CH], pf3[:, :, 0:CH - 1],
                                    pp3[:, :, 1:CH], op=ALU.add)
                                nc.vector.tensor_copy(c13[:, :, 0:1],
                                                      pp3[:, :, 0:1])
                                c2 = apool.tile([128, BW], F16, name=f"c2{qk}",
                                                tag="t1")
                                nc.vector.tensor_tensor(c2[:], c1[:], ps_[:],
                                                        op=ALU.subtract)
                                I = scan(c2, "I")
                                t2 = apool.tile([128, BW], F16, name=f"t2{qk}",
                                                tag="ts")
                                nc.vector.tensor_tensor(t2[:], xb[:], I[:],
                                                        op=ALU.add)
                                # free-dim permute (s,l)->(l,s) on Scalar
                                o2 = apool.tile([128, BW], F16, name=f"o2{qk}",
                                                tag="o2")
                                nc.scalar.copy(
                                    o2[:].rearrange("p (l s) -> p l s", s=SB),
                                    t2[:].rearrange("p (s l) -> p l s", l=CH))
                                store_eng.dma_start(
                                    dram[er, 0:CH, sr],
                                    o2[:].rearrange("p (l s) -> p l s", s=SB))

                            half("q", 0, 2, 4, qdram, nc.scalar)
                            half("k", 1, 3, 5, kdram, nc.sync)


            # ================= B-terms =================
            def emit_b():
                Bqp, Bkp = [], []  # fp16 [128, S] per fm, proj-space
                for qk, w_sb, B_eb, bj, dst in (
                        ("q", wq_sb, bq_eb, 0, Bqp),
                        ("k", wk_sb, bk_eb, 1, Bkp)):
                    for fm in range(EB):
                        fr = slice(fm * 128, (fm + 1) * 128)
                        acc = ps_a.tile([128, S], F32, name=f"psB{qk}{fm}",
                                        tag="ps_mm")
                        for eb in range(EB):
                            nc.tensor.matmul(acc[:], w_sb[eb][:, fr],
                                             B_eb[eb][:],
                                             start=(eb == 0), stop=(eb == EB - 1))
                        o = cpool.tile([128, S], F16, name=f"B{qk}p_{fm}")
                        nc.vector.tensor_scalar_add(
                            o[:], acc[:],
                            biasc_sb[:, 4 * fm + bj:4 * fm + bj + 1])
                        dst.append(o)
                return Bqp, Bkp

            def attn_stage1(n0, qt2, kt2, xt2, Bqp, Bkp, apool):
                """proj + v-proj + pass1 softmax stats + lse -> state dict."""
                T = cfg.tune
                NP = NPAIR

                def proj(w_sb, src2, Bp, nm):
                    outt = []
                    for fm in range(EB):
                        fr = slice(fm * 128, (fm + 1) * 128)
                        acc = ps_a.tile([128, NP * S], F32, name=f"ps{nm}{fm}",
                                        tag="ps_mm")
                        for eb in range(EB):
                            nc.tensor.matmul(acc[:], w_sb[eb][:, fr],
                                             src2[eb][:],
                                             start=(eb == 0), stop=(eb == EB - 1))
                        o = apool.tile([128, NP * S], F16, name=f"{nm}_{fm}",
                                       tag="qkp", bufs=T["qkp"])
                        for j in range(NP):
                            js = slice(j * S, (j + 1) * S)
                            nc.vector.tensor_tensor(o[:, js], acc[:, js],
                                                    Bp[fm][:], op=ALU.add)
                        outt.append(o)
                    return outt

                qp = proj(wq_sb, qt2, Bqp, "qp")
                kp = proj(wk_sb, kt2, Bkp, "kp")

                def hv(p, h, j):
                    return p[h][:, j * S:(j + 1) * S]

                vp = [[None] * NST for _ in range(NP)]
                for j in range(NP):
                    for st in range(NST):
                        scols = slice(j * S + st * 128, j * S + st * 128 + STW)
                        acc = ps_a.tile([STW, E], F32, name=f"psv{j}{st}",
                                        tag="ps_mm")
                        for eb in range(EB):
                            nc.tensor.matmul(acc[:], xt2[eb][:, scols],
                                             wv_sb[eb][:],
                                             start=(eb == 0), stop=False)
                        nc.tensor.matmul(acc[:], ones_row[:1, :STW],
                                         biasv_sb[:1, :], start=False, stop=True)
                        o = apool.tile([STW, E], F16, name=f"vp{j}{st}",
                                       tag="vp", bufs=T["vp"])
                        nc.vector.tensor_copy(o[:], acc[:])
                        vp[j][st] = o

                # pass 1: (s,t) scores -> -max, den (both j in one den tile)
                nmax_c = []
                den_pair = apool.tile([STW, NP * 2 * H], F32, name="denp",
                                      tag="denp", bufs=3)
                escr = apool.tile([STW, S], F16, name="escr", tag="escr", bufs=2)
                for j in range(NP):
                    nm_ = apool.tile([STW, 2 * H], F32, name=f"nmaxc{j}",
                                     tag="nmaxc", bufs=4)
                    nmax_c.append(nm_)
                    for st in range(NST):
                        scols = slice(st * 128, st * 128 + STW)
                        for h in range(H):
                            c = h * NST + st
                            accs = ps_b.tile([STW, S], F32, name=f"ps1{j}{st}{h}",
                                             tag="ps_sc")
                            nc.tensor.matmul(accs[:], hv(qp, h, j)[:, scols],
                                             hv(kp, h, j), start=True, stop=True)
                            nc.vector.tensor_reduce(
                                nm_[:, c:c + 1], accs[:], axis=AX.X,
                                op=ALU.max, negate=True)
                            nc.scalar.activation(
                                escr[:], accs[:], ACTF.Exp,
                                bias=nm_[:, c:c + 1], scale=1.0,
                                accum_out=den_pair[:, j * 2 * H + c:j * 2 * H + c + 1])
                # ONE Ln for the whole pair
                ln_pair = apool.tile([STW, NP * 2 * H], F32, name="lnp",
                                     tag="lnp", bufs=3)
                nc.scalar.activation(ln_pair[:], den_pair[:], ACTF.Ln)
                lseflat = []
                for j in range(NP):
                    lse32 = apool.tile([STW, 2 * H], F32, name=f"lse32{j}",
                                       tag="lse32", bufs=4)
                    nc.vector.tensor_tensor(
                        lse32[:], nmax_c[j][:],
                        ln_pair[:, j * 2 * H:(j + 1) * 2 * H],
                        op=ALU.subtract)  # -(max+ln den)
                    pk = apool.tile([STW, 4 * H], F16, name=f"lsepack{j}",
                                    tag="lsepack", bufs=4)
                    nc.vector.tensor_copy(pk[:, 0:2 * H], lse32[:])
                    resid = apool.tile([STW, 2 * H], F32, name=f"resid{j}",
                                       tag="resid", bufs=4)
                    nc.vector.tensor_tensor(resid[:], lse32[:], pk[:, 0:2 * H],
                                            op=ALU.subtract)
                    nc.vector.tensor_copy(pk[:, 2 * H:4 * H], resid[:])
                    lf = apool.tile([1, STW * 4 * H], F16, name=f"lseflat{j}",
                                    tag="lseflat", bufs=T["lseflat"])
                    nc.sync.dma_start(
                        lf[:].rearrange("o (s r) -> o s r", r=4 * H), pk[:])
                    lseflat.append(lf)
                return dict(n0=n0, qp=qp, kp=kp, vp=vp, lseflat=lseflat, hv=hv)

            def attn_stage2(stt, apool):
                """pass2 + attn@V + out-projection for a stage1'd pair."""
                T = cfg.tune
                NP = NPAIR
                n0, qp, kp, vp, lseflat, hv = (stt["n0"], stt["qp"], stt["kp"],
                                               stt["vp"], stt["lseflat"],
                                               stt["hv"])
                PT = [[[None] * NST for _ in range(H)] for _ in range(NP)]
                for j in range(NP):
                    lse_rs = lseflat[j][:].rearrange("o (s r) -> o r s",
                                                     r=4 * H)
                    for grp in range(2):
                        accs2 = []
                        for h2 in range(2):
                            h = grp * 2 + h2
                            for tt in range(NST):
                                tcols = slice(tt * 128, tt * 128 + STW)
                                acc = ps_b.tile([STW, S], F32,
                                                name=f"ps2{j}{h}{tt}",
                                                tag="ps_sc")
                                nc.tensor.matmul(acc[:], hv(kp, h, j)[:, tcols],
                                                 hv(qp, h, j),
                                                 start=True, stop=False)
                                accs2.append((acc, h, tt))
                        for acc, h, tt in accs2:
                            for part in range(2):
                                r0 = part * 2 * H + h * NST
                                nc.tensor.matmul(
                                    acc[:], ones_row[:1, :STW],
                                    lse_rs[:, r0:r0 + NST, :],
                                    start=False, stop=(part == 1))
                            p = apool.tile([STW, S], F16, name=f"PT{j}{h}{tt}",
                                           tag="PT", bufs=T["PT"])
                            nc.scalar.activation(p[:], acc[:], ACTF.Exp)
                            PT[j][h][tt] = p

                osc = []
                for h in range(H):
                    hr = slice(h * HD, (h + 1) * HD)
                    acc = ps_t.tile([HD, NP * S], F32, name=f"pso{h}",
                                    tag="ps_oo")
                    for j in range(NP):
                        js = slice(j * S, (j + 1) * S)
                        for tt in range(NST):
                            nc.tensor.matmul(acc[:, js], vp[j][tt][:, hr],
                                             PT[j][h][tt][:],
                                             start=(tt == 0), stop=(tt == NST - 1))
                    o = apool.tile([HD, NP * S], F16, name=f"osc{h}", tag="osc",
                                   bufs=T["osc"])
                    nc.vector.tensor_copy(o[:], acc[:])
                    osc.append(o)

                for gm in range(EB):
                    gr = slice(gm * 128, (gm + 1) * 128)
                    acc = ps_a.tile([128, NP * S], F32, name=f"psout{gm}",
                                    tag="ps_mm")
                    for fm in range(EB):
                        nc.tensor.matmul(acc[:], wo_sb[fm][:, gr], osc[fm][:],
                                         start=(fm == 0), stop=(fm == EB - 1))
                    o = apool.tile([128, NP * S], F16, name=f"oo{gm}", tag="oo",
                                   bufs=T["oo"])
                    nc.vector.tensor_scalar_add(
                        o[:], acc[:], biasc_sb[:, 4 * gm + 3:4 * gm + 4])
                    nc.scalar.dma_start(
                        out[gr, n0:n0 + NP, :],
                        o[:].rearrange("p (j s) -> p j s", j=NP))

            def emit_attn_all(Bqp, Bkp):
                with (
                    tc.tile_pool(name="dpool", bufs=cfg.tune["dpool"]) as dpool,
                    tc.tile_pool(name="attn", bufs=2) as apool,
                ):
                    NMAX = cfg.nmax if not cfg.skip_attn else 0
                    assert NMAX % NPAIR == 0
                    prev = None
                    for n0 in range(0, NMAX, NPAIR):
                        qt2, kt2, xt2 = [], [], []
                        nsl = slice(n0, n0 + NPAIR)
                        for eb in range(EB):
                            er = slice(eb * 128, (eb + 1) * 128)
                            t = dpool.tile([128, NPAIR * S], F16, name=f"qt{eb}",
                                           tag=f"qt{eb}")
                            nc.sync.dma_start(t[:], qdram[er, nsl, :])
                            qt2.append(t)
                            t = dpool.tile([128, NPAIR * S], F16, name=f"kt{eb}",
                                           tag=f"kt{eb}")
                            nc.sync.dma_start(t[:], kdram[er, nsl, :])
                            kt2.append(t)
                            t = dpool.tile([128, NPAIR * S], F16, name=f"xt{eb}",
                                           tag=f"xt{eb}")
                            nc.sync.dma_start(t[:], xattn[er, nsl, :])
                            xt2.append(t)
                        cur = attn_stage1(n0, qt2, kt2, xt2, Bqp, Bkp, apool)
                        if prev is not None:
                            attn_stage2(prev, apool)
                        prev = cur
                    if prev is not None:
                        attn_stage2(prev, apool)

            for _rep in range(cfg.reps):
                if not cfg.skip_band:
                    emit_band()
                Bqp, Bkp = emit_b()
                emit_attn_all(Bqp, Bkp)

    nc.finalize()
    return nc


# ============================================================
# host side
# ============================================================

def prep_inputs(cfg: Cfg, x, a, b, c, d, in_proj_w, in_proj_b, out_w, out_b):
    S, L, E, NC, CH, OFF = cfg.S, cfg.L, cfg.E, cfg.NC, cfg.CH, cfg.OFF
    f32, f16 = np.float32, np.float16
    x = np.asarray(x, f32)
    xg = np.ascontiguousarray(x.transpose(2, 0, 1))     # (E, S, L)
    hd = cfg.HD
    scl = 1.0 / math.sqrt(hd)
    wq = np.ascontiguousarray(in_proj_w[:E].T * scl).astype(f16)
    wk = np.ascontiguousarray(in_proj_w[E:2 * E].T).astype(f16)
    wv = np.ascontiguousarray(in_proj_w[2 * E:].T).astype(f16)
    wo = np.ascontiguousarray(out_w.T).astype(f16)
    bq = in_proj_b[:E] * scl
    bk = in_proj_b[E:2 * E]
    bv = in_proj_b[2 * E:]
    bo = out_b
    biasv = np.asarray(bv, f16).reshape(1, E)
    # last-column fwd weights (per core below)
    biasc = np.ascontiguousarray(
        np.stack([bq, bk, bv, bo]).astype(f32).T)       # (E, 4)
    ident = np.eye(128, dtype=f16)

    gate = np.ones((128, cfg.SB * CH), f16)
    gate[:, ::CH] = 0.0

    # boundary chunk totals: T[kind][j][e,s] = sum_{l in chunk j} x[s,l,e]*w[l,e]
    xr = x.reshape(S, NC, CH, E)
    Tt = {}
    for nmw, w in (("a", a), ("b", b), ("c", c), ("d", d)):
        Tt[nmw] = np.einsum("sjle,jle->jes", xr,
                            np.asarray(w, f32).reshape(NC, CH, E),
                            optimize=True)

    in_maps = []
    for k in range(NC):
        chk = slice(CH * k, CH * (k + 1))
        xbandc = np.ascontiguousarray(xg[:, :, chk]).astype(f16)
        xattnc = np.ascontiguousarray(
            xg[:, :, chk].transpose(0, 2, 1)).astype(f16)
        if k >= OFF:
            pf = slice(CH * (k - OFF), CH * (k - OFF + 1))
            xpc = np.ascontiguousarray(xg[:, :, pf]).astype(f16)
            w1 = -a[pf].astype(f32)
            w2 = -b[pf].astype(f32)
        else:
            st = CH * (k + OFF) - 1
            xpc = np.zeros((E, S, CH), f16)
            xpc[:, :, 1:] = xg[:, :, st + 1:st + CH]
            w1 = np.zeros((CH, E), f32)
            w1[1:] = c[st + 1:st + CH]
            w2 = np.zeros((CH, E), f32)
            w2[1:] = d[st + 1:st + CH]
        wbandc = np.ascontiguousarray(
            np.stack([a[chk], b[chk], c[chk], d[chk], w1, w2])
            .transpose(0, 2, 1)).astype(f16)            # (6, E, CH)
        jA = slice(max(0, k - OFF), k)
        jC = slice(k, min(k + OFF - 1, NC - 1) + 1)
        bqc = (Tt["a"][jA].sum(0) + Tt["c"][jC].sum(0)).astype(f16)
        bkc = (Tt["b"][jA].sum(0) + Tt["d"][jC].sum(0)).astype(f16)
        in_maps.append(dict(
            xband=xbandc, xattn=xattnc, xp=xpc,
            wband=wbandc, gate_in=gate, bqin=bqc, bkin=bkc,
            wq=wq, wk=wk, wv=wv, wo=wo, biasv=biasv, biasc=biasc,
            ident_in=ident,
        ))
    return in_maps


_CACHE = {}


def run(cfg: Cfg, inputs, core_ids=None, **kw):
    key = cfg.key()
    if key not in _CACHE:
        _CACHE[key] = build_nc(cfg)
    nc = _CACHE[key]
    in_maps = prep_inputs(
        cfg, inputs["x"], inputs["a"], inputs["b"], inputs["c"], inputs["d"],
        inputs["in_proj_w"], inputs["in_proj_b"], inputs["out_w"], inputs["out_b"])
    res = run_bass_kernel_spmd(nc, in_maps, core_ids or list(range(cfg.NC)), **kw)
    S, L, E, CH = cfg.S, cfg.L, cfg.E, cfg.CH
    full = np.empty((S, L, E), np.float32)
    for k in range(cfg.NC):
        # out is (E, CH, S) fp16
        full[:, CH * k:CH * (k + 1), :] = \
            res.results[k]["out"].astype(np.float32).transpose(2, 1, 0)
    return full, res


def kernel(**inputs) -> np.ndarray:
    assert int(inputs["n1"]) == 256 and int(inputs["n2"]) == 256
    cfg = Cfg()
    out, _ = run(cfg, inputs)
    return out
